# revision 24
# baseline (speedup 1.0000x reference)
"""CSWin attention Trainium2 kernel (v6 — flipped AV, split LePE, XBAR
attention transposes, in-loop concat assembly).

Shapes (hardcoded): B=8, H=W=64, N=4096, C=512, 8 heads (4 horizontal-stripe,
4 vertical-stripe), head_dim=64, stripe width SPLIT=8.

Sharding: data-parallel over batch B across the 8 NeuronCores (1 image/core).

Per-core structure (all matmuls bf16, fp32 PSUM):
  - prologue: batched f32 DMA loads (first x chunk prefetched ahead of the
    weights); f32->bf16 casts split DVE/ScalarE; x/W transposes as PE
    [128,128] transposes through bf16 PSUM (kept on PE: XBAR versions
    serialize on the issuing sequencer and add latency to the qkv
    pipeline); per-512-token chunk: 12 qkv matmuls + PSUM->SBUF bias
    copies (h-half on ScalarE via Identity+per-partition bias, v-half
    scattered col-major on DVE).
  - attention: 32 head-pairs in two interleaved streams (h / v), LePE AND
    scores both software-pipelined one pair ahead.  LePE split: center +
    4 corner taps as diagonal matmuls on PE (center first: a PSUM zero
    region gets exactly one start=True and it must cover the bank), 4
    edge taps as a DVE in-place scalar_tensor_tensor chain; one DVE op
    combines PSUM + chain + bias into bf16.  v_lepe goes token-major via
    XBAR DMA transpose (14 ns per 16x128 tile on the idle DMA engines).
    AV is flipped: out[tq, u*65] packs 4 tq-chunks in one PSUM bank and
    uses all 128 PE output partitions (65 cols/matmul instead of 512);
    interleaved 1-col ones-matmuls produce the softmax denominator in a
    per-partition column.  Normalization: batched DVE reciprocal [128,4]
    + one 0-stride-broadcast tensor_tensor per head-stripe into a small
    token-major collect tile.  Assembly runs in-loop per pair: XBAR
    transpose to channel-major concatT (v-half through a col-major
    staging tile + DVE scatter-copy for the global token order).
  - proj: 32 chunks; bias via K=1 ones-matmul on even chunks (ScalarE
    copy-out) and folded into the DVE PSUM->SBUF add on odd chunks.
"""

import numpy as np

import concourse.bass as bass
import concourse.bacc as bacc
import concourse.mybir as mybir
from concourse import bass_utils
from concourse.tile import TileContext
from concourse.masks import make_identity

F32 = mybir.dt.float32
BF16 = mybir.dt.bfloat16
ALU = mybir.AluOpType
ACT = mybir.ActivationFunctionType

B = 8
H = 64
W = 64
N = H * W          # 4096
C = 512
NH = 8             # heads
HD = 64            # head dim
SP = 8             # stripe width
NS = 8             # stripes per direction
SCALE = HD ** -0.5

# LePE tap split: PE gets center (full width, owns the PSUM zero region),
# all 4 corners and the k=1 edge; DVE chain gets taps 3, 5, 7 (first one
# initializes the accumulator; its complement is memset to 0).
PE_TAPS = (4, 0, 2, 6, 8, 1)
DVE_TAPS = (3, 5, 7)

_CACHE = {}


def _win(t, off, nr, nx):
    return bass.AP(tensor=t.tensor, offset=t.offset + off,
                   ap=[t.ap[0], [64, nr], [1, nx]])


def _geom(k):
    dr, dc = k // 3 - 1, k % 3 - 1
    r0 = max(0, -dr)
    nr = 8 - abs(dr)
    x0 = max(0, -dc)
    nx = 64 - abs(dc)
    return r0 * 64 + x0, (r0 + dr) * 64 + (x0 + dc), nr, nx


def _build_nc():
    nc = bacc.Bacc("TRN2", target_bir_lowering=False, debug=False)

    x_d = nc.dram_tensor("x", (N, C), F32, kind="ExternalInput").ap()
    wqkv_d = nc.dram_tensor("wqkv", (3 * C, C), F32, kind="ExternalInput").ap()
    bqkv_d = nc.dram_tensor("bqkv", (3 * C,), F32, kind="ExternalInput").ap()
    wproj_d = nc.dram_tensor("wproj", (C, C), F32, kind="ExternalInput").ap()
    bproj_d = nc.dram_tensor("bproj", (C,), F32, kind="ExternalInput").ap()
    lhw_d = nc.dram_tensor("lepe_h_w", (3, 3, 1, HD), F32, kind="ExternalInput").ap()
    lhb_d = nc.dram_tensor("lepe_h_b", (HD,), F32, kind="ExternalInput").ap()
    lvw_d = nc.dram_tensor("lepe_v_w", (3, 3, 1, HD), F32, kind="ExternalInput").ap()
    lvb_d = nc.dram_tensor("lepe_v_b", (HD,), F32, kind="ExternalInput").ap()
    y_d = nc.dram_tensor("y", (N, C), F32, kind="ExternalOutput").ap()

    with TileContext(nc) as tc:
        _emit(nc, tc, x_d, wqkv_d, bqkv_d, wproj_d, bproj_d,
              lhw_d, lhb_d, lvw_d, lvb_d, y_d)
    nc.compile()
    return nc


def _emit(nc, tc, x_d, wqkv_d, bqkv_d, wproj_d, bproj_d,
          lhw_d, lhb_d, lvw_d, lvb_d, y_d):
    import contextlib
    ctx = contextlib.ExitStack()
    with ctx:
        persist = ctx.enter_context(tc.tile_pool(name="persist", bufs=1))
        qkv_pool = ctx.enter_context(tc.tile_pool(name="qkvT", bufs=1))
        concat_pool = ctx.enter_context(tc.tile_pool(name="concat", bufs=1))

        from concourse import library_config
        nc.gpsimd.load_library(library_config.proxy)

        # ---------------- constants ----------------
        id64 = persist.tile([64, 64], BF16, tag="id64")
        make_identity(nc, id64)
        ones_col = persist.tile([128, 1], BF16, tag="ones_col")
        nc.vector.memset(ones_col, 1.0)
        ones_row = persist.tile([1, 128], BF16, tag="ones_row")
        nc.vector.memset(ones_row, 1.0)
        id128 = persist.tile([128, 128], BF16, tag="id128")
        make_identity(nc, id128)

        qkvT = [qkv_pool.tile([128, N], BF16, name=f"qkvT{jt}", tag=f"qkvT{jt}")
                for jt in range(12)]
        concatT = [concat_pool.tile([128, N], BF16, name=f"concatT{fc}",
                                    tag=f"concatT{fc}") for fc in range(4)]

        # W layouts (single wide tiles, chunk-major):
        #   wq_big[cp, cc*1536 + jt*128 + s]   (lhsT chunks for qkv)
        #   wp_big[fp, fc*512 + e]             (rhs chunks for proj)
        wq_big = persist.tile([128, 4 * 1536], BF16, tag="wq_big")
        wp_big = persist.tile([128, 4 * 512], BF16, tag="wp_big")

        # ---------------- prologue ----------------
        with tc.tile_pool(name="xload", bufs=3) as xload, \
             tc.tile_pool(name="xcast", bufs=3) as xcast, \
             tc.tile_pool(name="xtg", bufs=3) as xtg_pool, \
             tc.tile_pool(name="qkv_psum", bufs=4, space="PSUM") as qkv_psum, \
             tc.tile_pool(name="w_psum", bufs=3, space="PSUM") as w_psum:

            def pe_transpose_block(wrow_bf, out_tile, base, blk_stride):
                """4x4 [128,128] PE transposes: out[:, base + cc*blk_stride
                + j*128 + s] = wrow_bf[:, j, cc*128+s].T, via bf16 PSUM."""
                for cc in range(4):
                    ps = w_psum.tile([128, 512], BF16, tag="xps")
                    for j in range(4):
                        nc.tensor.transpose(
                            ps[:, j * 128:(j + 1) * 128],
                            wrow_bf[:, j, cc * 128:(cc + 1) * 128], id128)
                    dst = bass.AP(
                        tensor=out_tile.tensor,
                        offset=out_tile.offset + base + cc * blk_stride,
                        ap=[out_tile.ap[0], [1, 512]])
                    if cc % 2 == 0:
                        nc.scalar.activation(dst, ps, ACT.Copy)
                    else:
                        nc.vector.tensor_copy(dst, ps)

            # prefetch the first x chunk before the W loads
            xrow0 = xload.tile([128, 4, C], F32, tag="xrow")
            nc.sync.dma_start(
                out=xrow0,
                in_=x_d.rearrange("(a p) c -> p a c", p=128)[:, 0:4, :])

            # --- Wqkv ---
            for jg in range(3):
                wrow = xload.tile([128, 4, C], F32, tag="xrow")
                nc.sync.dma_start(
                    out=wrow,
                    in_=wqkv_d.rearrange("(a p) c -> p a c", p=128)[
                        :, jg * 4:(jg + 1) * 4, :])
                wrow_bf = xcast.tile([128, 4, C], BF16, tag="xrow_bf")
                for j in range(4):
                    if j % 2 == 0:
                        nc.vector.tensor_copy(wrow_bf[:, j], wrow[:, j])
                    else:
                        nc.scalar.activation(wrow_bf[:, j], wrow[:, j], ACT.Copy)
                pe_transpose_block(wrow_bf, wq_big, jg * 512, 1536)

            # --- biases ---
            bqkv_sb = persist.tile([128, 12], F32, tag="bqkv")
            nc.sync.dma_start(out=bqkv_sb,
                              in_=bqkv_d.rearrange("(a p) -> p a", p=128))
            # --- x chunks: load -> cast -> PE transpose -> qkv matmuls ---
            for tg in range(8):
                if tg == 0:
                    xrow = xrow0
                else:
                    xrow = xload.tile([128, 4, C], F32, tag="xrow")
                    nc.sync.dma_start(
                        out=xrow,
                        in_=x_d.rearrange("(a p) c -> p a c", p=128)[
                            :, tg * 4:(tg + 1) * 4, :])
                xrow_bf = xcast.tile([128, 4, C], BF16, tag="xrow_bf")
                for j in range(4):
                    if j % 2 == 0:
                        nc.vector.tensor_copy(xrow_bf[:, j], xrow[:, j])
                    else:
                        nc.scalar.activation(xrow_bf[:, j], xrow[:, j], ACT.Copy)
                # xTg[cp, cc*512 + j*128 + tsub] for this token chunk
                xTg = xtg_pool.tile([128, 2048], BF16, tag="xTg", name="xTg")
                pe_transpose_block(xrow_bf, xTg, 0, 512)
                for jt in range(12):
                    vhalf = (jt % 4) >= 2
                    ps = qkv_psum.tile([128, 512], F32, tag="qkvps")
                    for cc in range(4):
                        nc.tensor.matmul(
                            ps,
                            wq_big[:, cc * 1536 + jt * 128:
                                   cc * 1536 + jt * 128 + 128],
                            xTg[:, cc * 512:(cc + 1) * 512],
                            start=(cc == 0), stop=(cc == 3))
                    if vhalf:
                        out_ap = bass.AP(
                            tensor=qkvT[jt].tensor,
                            offset=qkvT[jt].offset + 8 * tg,
                            ap=[qkvT[jt].ap[0], [1, 8], [64, 64]])
                        nc.vector.tensor_scalar_add(
                            out_ap, ps, bqkv_sb[:, jt:jt + 1])
                    else:
                        nc.scalar.activation(
                            qkvT[jt][:, tg * 512:(tg + 1) * 512], ps,
                            ACT.Identity, bias=bqkv_sb[:, jt:jt + 1])

            # --- Wproj ---
            wrow = xload.tile([128, 4, C], F32, tag="xrow")
            nc.sync.dma_start(
                out=wrow, in_=wproj_d.rearrange("(a p) c -> p a c", p=128))
            wrow_bf = xcast.tile([128, 4, C], BF16, tag="xrow_bf")
            for j in range(4):
                if j % 2 == 0:
                    nc.vector.tensor_copy(wrow_bf[:, j], wrow[:, j])
                else:
                    nc.scalar.activation(wrow_bf[:, j], wrow[:, j], ACT.Copy)
            pe_transpose_block(wrow_bf, wp_big, 0, 512)

            bproj_row = persist.tile([1, C], F32, tag="bproj_row")
            nc.sync.dma_start(out=bproj_row,
                              in_=bproj_d.rearrange("(a e) -> a e", a=1))
            bproj_sb = persist.tile([1, C], BF16, tag="bproj_sb")
            nc.vector.tensor_copy(bproj_sb, bproj_row)
            bproj_bc = persist.tile([128, C], F32, tag="bproj_bc")
            nc.gpsimd.partition_broadcast(bproj_bc, bproj_row)
            lepe_b = []
            for name, d in (("lhb", lhb_d), ("lvb", lvb_d)):
                t = persist.tile([128, 1], F32, name=name, tag=name)
                nc.sync.dma_start(out=t[0:64, :],
                                  in_=d.rearrange("(p a) -> p a", a=1))
                nc.sync.dma_start(out=t[64:128, :],
                                  in_=d.rearrange("(p a) -> p a", a=1))
                lepe_b.append(t)

            # --- LePE weights ---
            lepw = []
            diags = []
            for half, wsrc in ((0, lhw_d), (1, lvw_d)):
                w9 = xload.tile([9, 64], F32, tag="w9")
                nc.sync.dma_start(out=w9,
                                  in_=wsrc.rearrange("a b c d -> (a b c) d"))
                w9_bf = xcast.tile([9, 64], BF16, tag="w9bf")
                nc.vector.tensor_copy(w9_bf, w9)
                ps = w_psum.tile([64, 9], BF16, tag="wTps", bufs=1)
                nc.tensor.transpose(ps, w9_bf, id64[0:9, 0:9])
                wT = persist.tile([128, 9], F32, tag=f"wT{half}")
                nc.vector.tensor_copy(wT[0:64, :], ps)
                nc.sync.dma_start(out=wT[64:128, :], in_=wT[0:64, :])
                wis = []
                for k in range(9):
                    dr, dc = k // 3 - 1, k % 3 - 1
                    wis.append((dr + 1) * 3 + (dc + 1) if half == 0
                               else (dc + 1) * 3 + (dr + 1))
                wic = wis[4]
                nc.vector.tensor_scalar_add(wT[:, wic:wic + 1],
                                            wT[:, wic:wic + 1], 1.0)
                lepw.append((wT, wis))
                dh = {}
                for k in PE_TAPS:
                    dt = persist.tile([128, 128], BF16, tag=f"diag{half}_{k}")
                    nc.vector.tensor_scalar_mul(dt, id128,
                                                wT[:, wis[k]:wis[k] + 1])
                    dh[k] = dt
                diags.append(dh)


        # ---------------- attention ----------------
        pairs = [(half, s, hp)
                 for half in range(2) for s in range(NS) for hp in range(2)]

        with tc.tile_pool(name="sc_psum", bufs=2, space="PSUM") as sc_psum, \
             tc.tile_pool(name="lp_psum", bufs=2, space="PSUM") as lp_psum, \
             tc.tile_pool(name="oa_psum", bufs=2, space="PSUM") as oa_psum, \
             tc.tile_pool(name="att", bufs=8) as att, \
             tc.tile_pool(name="lepe_sb", bufs=3) as lepe_sb, \
             tc.tile_pool(name="norm_sb", bufs=6) as norm_sb, \
             tc.tile_pool(name="ttp", bufs=4) as ttp_pool, \
             tc.tile_pool(name="vcm", bufs=3) as vcm_pool:

            def vsp_of(pi):
                half, s, hp = pairs[pi]
                return qkvT[8 + half * 2 + hp][:, s * 512:s * 512 + 512]

            def emit_pe_taps(pi):
                """center + 5 taps as diagonal matmuls in PSUM."""
                half, s, hp = pairs[pi]
                vsp = vsp_of(pi)
                vl = lp_psum.tile([128, 512], F32, tag="lps", name="vl")
                nc.tensor.matmul(vl, diags[half][4], vsp,
                                 start=True, stop=False, skip_group_check=True)
                for ki, k in enumerate(PE_TAPS[1:]):
                    o_off, i_off, nr, nx = _geom(k)
                    nc.tensor.matmul(
                        _win(vl, o_off, nr, nx), diags[half][k],
                        _win(vsp, i_off, nr, nx),
                        start=False, stop=(ki == len(PE_TAPS) - 2),
                        skip_group_check=True)
                return vl

            def emit_dve_chain(pi):
                """taps 3,5,7 in-place on DVE; first tap initializes."""
                half, s, hp = pairs[pi]
                vsp = vsp_of(pi)
                wT, wis = lepw[half]
                acc = lepe_sb.tile([128, 512], F32, tag="acc", name="acc")
                k0 = DVE_TAPS[0]
                o_off, i_off, nr, nx = _geom(k0)
                nc.vector.tensor_scalar_mul(
                    _win(acc, o_off, nr, nx), _win(vsp, i_off, nr, nx),
                    wT[:, wis[k0]:wis[k0] + 1])
                # complement of tap 3's window ([8 rows, cols 1..63]): col 0
                nc.vector.memset(
                    bass.AP(tensor=acc.tensor, offset=acc.offset,
                            ap=[acc.ap[0], [64, 8], [1, 1]]), 0.0)
                for k in DVE_TAPS[1:]:
                    o_off, i_off, nr, nx = _geom(k)
                    nc.vector.scalar_tensor_tensor(
                        _win(acc, o_off, nr, nx), _win(vsp, i_off, nr, nx),
                        wT[:, wis[k]:wis[k] + 1], _win(acc, o_off, nr, nx),
                        ALU.mult, ALU.add)
                return acc

            def emit_combine(pi, vl, acc):
                half, s, hp = pairs[pi]
                vlsb = lepe_sb.tile([128, 512], BF16, tag="vlsb", name="vlsb")
                nc.vector.scalar_tensor_tensor(
                    vlsb, vl, lepe_b[half], acc, ALU.add, ALU.add)
                return vlsb

            def emit_vna(vlsb):
                vna = lepe_sb.tile([128, 4, 128], BF16, tag="vna", name="vna")
                nc.sync.dma_start_transpose(vna, vlsb)
                return vna

            def emit_scores(pi):
                half, s, hp = pairs[pi]
                tok0 = s * 512
                jt_off = half * 2 + hp
                esbs = []
                for hh in range(2):
                    esbs.append(att.tile([128, 2048], BF16, tag="esb",
                                         name="esb"))
                for sh in range(2):
                    for hh in range(2):
                        pbase = hh * 64
                        qs = qkvT[jt_off][pbase:pbase + 64, tok0:tok0 + 512]
                        ks = qkvT[4 + jt_off][pbase:pbase + 64, tok0:tok0 + 512]
                        sps = sc_psum.tile([128, 1024], F32, tag="sps",
                                           name="sps")
                        for jj in range(2):
                            jc = 2 * sh + jj
                            nc.tensor.matmul(
                                sps[:, jj * 512:(jj + 1) * 512],
                                ks[:, jc * 128:(jc + 1) * 128], qs,
                                start=True, stop=True)
                        nc.scalar.activation(
                            esbs[hh][:, sh * 1024:(sh + 1) * 1024], sps,
                            ACT.Exp, bias=0.0, scale=SCALE)
                return esbs

            def emit_av(hh, vna, esb):
                """flipped AV matmuls -> oa [128, 260] (one PSUM bank)."""
                oa = oa_psum.tile([128, 260], F32, tag="oa", name="oa")
                first = True
                for u in range(4):
                    for jc in range(4):
                        lhsT = esb[:, jc * 512 + u * 128:
                                   jc * 512 + u * 128 + 128]
                        nc.tensor.matmul(
                            oa[:, u * 65:u * 65 + 64], lhsT,
                            vna[:, jc, hh * 64:hh * 64 + 64],
                            start=first, stop=False, skip_group_check=True)
                        first = False
                        nc.tensor.matmul(
                            oa[:, u * 65 + 64:u * 65 + 65], lhsT, ones_col,
                            start=False, stop=(u == 3 and jc == 3),
                            skip_group_check=True)
                return oa

            def emit_norm(hh, oa, ttp):
                """batched reciprocal + 0-stride broadcast normalize into the
                pair's token-major collect tile (cols u*128 + 64*hh)."""
                rr = norm_sb.tile([128, 4], F32, tag="rr", name="rr")
                nc.vector.reciprocal(
                    rr, bass.AP(tensor=oa.tensor, offset=oa.offset + 64,
                                ap=[oa.ap[0], [65, 4]]))
                oa_data = bass.AP(tensor=oa.tensor, offset=oa.offset,
                                  ap=[oa.ap[0], [65, 4], [1, 64]])
                rr_b = bass.AP(tensor=rr.tensor, offset=rr.offset,
                               ap=[rr.ap[0], [1, 4], [0, 64]])
                out_ap = bass.AP(
                    tensor=ttp.tensor, offset=ttp.offset + 64 * hh,
                    ap=[ttp.ap[0], [128, 4], [1, 64]])
                nc.vector.tensor_tensor(out_ap, oa_data, rr_b, ALU.mult)

            def emit_assembly(pi, ttp):
                """pair's collect tile -> concatT (channel-major)."""
                half, s, hp = pairs[pi]
                fc = half * 2 + hp
                if half == 0:
                    out_ap = bass.AP(
                        tensor=concatT[fc].tensor,
                        offset=concatT[fc].offset + s * 512,
                        ap=[concatT[fc].ap[0], [128, 4], [1, 128]])
                    nc.sync.dma_start_transpose(out_ap, ttp)
                else:
                    vcm = vcm_pool.tile([128, 4, 128], BF16, tag="vcm",
                                        name="vcm")
                    nc.sync.dma_start_transpose(vcm, ttp)
                    out_ap = bass.AP(
                        tensor=concatT[fc].tensor,
                        offset=concatT[fc].offset + 8 * s,
                        ap=[concatT[fc].ap[0], [1, 8], [64, 64]])
                    nc.vector.tensor_copy(
                        out_ap, vcm.rearrange("p a b -> p (a b)"))

            # steady-state loop; LePE/vna AND scores pipelined 1 pair ahead
            streams = [list(range(0, 16)), list(range(16, 32))]
            vna_cur = [None, None]
            esb_cur = [None, None]
            for st in (0, 1):
                p0 = streams[st][0]
                vl = emit_pe_taps(p0)
                acc = emit_dve_chain(p0)
                vna_cur[st] = emit_vna(emit_combine(p0, vl, acc))
                esb_cur[st] = emit_scores(p0)
            nsteps = len(streams[0])
            for i in range(nsteps):
                p0, p1 = streams[0][i], streams[1][i]
                n0 = streams[0][i + 1] if i + 1 < nsteps else None
                n1 = streams[1][i + 1] if i + 1 < nsteps else None
                esbA, esbB = esb_cur
                # PE: NEXT pairs' scores first (their exps run this step).
                # DVE queue order: current norms lead (freeing oa banks for
                # PE), next-pair chains fill the middle, combines close.
                esbA_n = emit_scores(n0) if n0 is not None else None
                esbB_n = emit_scores(n1) if n1 is not None else None
                ttp0 = ttp_pool.tile([128, 512], BF16, tag="ttp", name="ttp")
                oa = emit_av(0, vna_cur[0], esbA[0])
                emit_norm(0, oa, ttp0)
                oa = emit_av(1, vna_cur[0], esbA[1])
                emit_norm(1, oa, ttp0)
                acc0 = emit_dve_chain(n0) if n0 is not None else None
                emit_assembly(p0, ttp0)
                vl0 = emit_pe_taps(n0) if n0 is not None else None
                ttp1 = ttp_pool.tile([128, 512], BF16, tag="ttp", name="ttp")
                oa = emit_av(0, vna_cur[1], esbB[0])
                emit_norm(0, oa, ttp1)
                oa = emit_av(1, vna_cur[1], esbB[1])
                emit_norm(1, oa, ttp1)
                acc1 = emit_dve_chain(n1) if n1 is not None else None
                emit_assembly(p1, ttp1)
                vl1 = emit_pe_taps(n1) if n1 is not None else None
                if n0 is not None:
                    vna_cur[0] = emit_vna(emit_combine(n0, vl0, acc0))
                if n1 is not None:
                    vna_cur[1] = emit_vna(emit_combine(n1, vl1, acc1))
                esb_cur = [esbA_n, esbB_n]

        # ---------------- proj ----------------
        with tc.tile_pool(name="pj_psum", bufs=4, space="PSUM") as pj_psum, \
             tc.tile_pool(name="pj", bufs=3) as pj:
            osb = None
            for tt in range(32):
                even = (tt % 2 == 0)
                ps = pj_psum.tile([128, C], F32, tag="pjps")
                for fcc in range(4):
                    nc.tensor.matmul(
                        ps, concatT[fcc][:, tt * 128:(tt + 1) * 128],
                        wp_big[:, fcc * 512:(fcc + 1) * 512],
                        start=(fcc == 0), stop=(fcc == 3),
                        skip_group_check=True)
                if even:
                    osb = pj.tile([128, 2, C], F32, tag="pjout", name="pjout")
                # bias folded into the DVE PSUM->SBUF add (no K=1 matmul)
                nc.vector.tensor_tensor(osb[:, tt % 2], ps, bproj_bc, ALU.add)
                if not even:
                    nc.sync.dma_start(
                        out=y_d.rearrange("(a p) c -> p a c", p=128)[
                            :, tt - 1:tt + 1, :],
                        in_=osb)


def _get_nc():
    if "nc" not in _CACHE:
        _CACHE["nc"] = _build_nc()
    return _CACHE["nc"]


def kernel(**inputs):
    x = np.asarray(inputs["x"], dtype=np.float32)
    names = {
        "wqkv": "Wqkv", "bqkv": "bqkv", "wproj": "Wproj", "bproj": "bproj",
        "lepe_h_w": "lepe_h_w", "lepe_h_b": "lepe_h_b",
        "lepe_v_w": "lepe_v_w", "lepe_v_b": "lepe_v_b",
    }
    shared = {k: np.ascontiguousarray(np.asarray(inputs[v], dtype=np.float32))
              for k, v in names.items()}
    nc = _get_nc()
    in_maps = []
    for b in range(B):
        m = dict(shared)
        m["x"] = np.ascontiguousarray(x[b])
        in_maps.append(m)
    res = bass_utils.run_bass_kernel_spmd(nc, in_maps, core_ids=list(range(B)))
    out = np.stack([res.results[b]["y"] for b in range(B)], axis=0)
    return out.astype(np.float32)


if __name__ == "__main__":
    rng = np.random.default_rng(0)
    ins = {
        "x": rng.standard_normal((B, N, C), dtype=np.float32),
        "Wqkv": rng.standard_normal((3 * C, C), dtype=np.float32) * C ** -0.5,
        "bqkv": np.zeros(3 * C, np.float32),
        "Wproj": rng.standard_normal((C, C), dtype=np.float32) * C ** -0.5,
        "bproj": np.zeros(C, np.float32),
        "lepe_h_w": rng.standard_normal((3, 3, 1, HD), dtype=np.float32) / 3,
        "lepe_h_b": np.zeros(HD, np.float32),
        "lepe_v_w": rng.standard_normal((3, 3, 1, HD), dtype=np.float32) / 3,
        "lepe_v_b": np.zeros(HD, np.float32),
        "H": np.int64(H), "W": np.int64(W),
    }
    out = kernel(**ins)
    print(out.shape, out.dtype)


# revision 25
# speedup vs baseline: 1.0126x; 1.0126x over previous
"""CSWin attention Trainium2 kernel (v6 — flipped AV, split LePE, XBAR
attention transposes, in-loop concat assembly).

Shapes (hardcoded): B=8, H=W=64, N=4096, C=512, 8 heads (4 horizontal-stripe,
4 vertical-stripe), head_dim=64, stripe width SPLIT=8.

Sharding: data-parallel over batch B across the 8 NeuronCores (1 image/core).

Per-core structure (all matmuls bf16, fp32 PSUM):
  - prologue: batched f32 DMA loads (first x chunk prefetched ahead of the
    weights); f32->bf16 casts split DVE/ScalarE; x/W transposes as PE
    [128,128] transposes through bf16 PSUM (kept on PE: XBAR versions
    serialize on the issuing sequencer and add latency to the qkv
    pipeline); per-512-token chunk: 12 qkv matmuls + PSUM->SBUF bias
    copies (h-half on ScalarE via Identity+per-partition bias, v-half
    scattered col-major on DVE).
  - attention: 32 head-pairs in two interleaved streams (h / v), LePE AND
    scores both software-pipelined one pair ahead.  LePE split: center +
    4 corner taps as diagonal matmuls on PE (center first: a PSUM zero
    region gets exactly one start=True and it must cover the bank), 4
    edge taps as a DVE in-place scalar_tensor_tensor chain; one DVE op
    combines PSUM + chain + bias into bf16.  v_lepe goes token-major via
    XBAR DMA transpose (14 ns per 16x128 tile on the idle DMA engines).
    AV is flipped: out[tq, u*65] packs 4 tq-chunks in one PSUM bank and
    uses all 128 PE output partitions (65 cols/matmul instead of 512);
    interleaved 1-col ones-matmuls produce the softmax denominator in a
    per-partition column.  Normalization: batched DVE reciprocal [128,4]
    + one 0-stride-broadcast tensor_tensor per head-stripe into a small
    token-major collect tile.  Assembly runs in-loop per pair: XBAR
    transpose to channel-major concatT (v-half through a col-major
    staging tile + DVE scatter-copy for the global token order).
  - proj: 32 chunks; bias via K=1 ones-matmul on even chunks (ScalarE
    copy-out) and folded into the DVE PSUM->SBUF add on odd chunks.
"""

import numpy as np

import concourse.bass as bass
import concourse.bacc as bacc
import concourse.mybir as mybir
from concourse import bass_utils
from concourse.tile import TileContext
from concourse.masks import make_identity

F32 = mybir.dt.float32
BF16 = mybir.dt.bfloat16
ALU = mybir.AluOpType
ACT = mybir.ActivationFunctionType

B = 8
H = 64
W = 64
N = H * W          # 4096
C = 512
NH = 8             # heads
HD = 64            # head dim
SP = 8             # stripe width
NS = 8             # stripes per direction
SCALE = HD ** -0.5

# LePE tap split: PE gets center (full width, owns the PSUM zero region),
# all 4 corners and the k=1 edge; DVE chain gets taps 3, 5, 7 (first one
# initializes the accumulator; its complement is memset to 0).
PE_TAPS = (4, 0, 2, 6, 8)
DVE_TAPS = (1, 3, 5, 7)

_CACHE = {}


def _win(t, off, nr, nx):
    return bass.AP(tensor=t.tensor, offset=t.offset + off,
                   ap=[t.ap[0], [64, nr], [1, nx]])


def _geom(k):
    dr, dc = k // 3 - 1, k % 3 - 1
    r0 = max(0, -dr)
    nr = 8 - abs(dr)
    x0 = max(0, -dc)
    nx = 64 - abs(dc)
    return r0 * 64 + x0, (r0 + dr) * 64 + (x0 + dc), nr, nx


def _build_nc():
    nc = bacc.Bacc("TRN2", target_bir_lowering=False, debug=False)

    x_d = nc.dram_tensor("x", (N, C), F32, kind="ExternalInput").ap()
    wqkv_d = nc.dram_tensor("wqkv", (3 * C, C), F32, kind="ExternalInput").ap()
    bqkv_d = nc.dram_tensor("bqkv", (3 * C,), F32, kind="ExternalInput").ap()
    wproj_d = nc.dram_tensor("wproj", (C, C), F32, kind="ExternalInput").ap()
    bproj_d = nc.dram_tensor("bproj", (C,), F32, kind="ExternalInput").ap()
    lhw_d = nc.dram_tensor("lepe_h_w", (3, 3, 1, HD), F32, kind="ExternalInput").ap()
    lhb_d = nc.dram_tensor("lepe_h_b", (HD,), F32, kind="ExternalInput").ap()
    lvw_d = nc.dram_tensor("lepe_v_w", (3, 3, 1, HD), F32, kind="ExternalInput").ap()
    lvb_d = nc.dram_tensor("lepe_v_b", (HD,), F32, kind="ExternalInput").ap()
    y_d = nc.dram_tensor("y", (N, C), F32, kind="ExternalOutput").ap()

    with TileContext(nc) as tc:
        _emit(nc, tc, x_d, wqkv_d, bqkv_d, wproj_d, bproj_d,
              lhw_d, lhb_d, lvw_d, lvb_d, y_d)
    nc.compile()
    return nc


def _emit(nc, tc, x_d, wqkv_d, bqkv_d, wproj_d, bproj_d,
          lhw_d, lhb_d, lvw_d, lvb_d, y_d):
    import contextlib
    ctx = contextlib.ExitStack()
    with ctx:
        persist = ctx.enter_context(tc.tile_pool(name="persist", bufs=1))
        qkv_pool = ctx.enter_context(tc.tile_pool(name="qkvT", bufs=1))
        concat_pool = ctx.enter_context(tc.tile_pool(name="concat", bufs=1))

        from concourse import library_config
        nc.gpsimd.load_library(library_config.proxy)

        # ---------------- constants ----------------
        id64 = persist.tile([64, 64], BF16, tag="id64")
        make_identity(nc, id64)
        ones_col = persist.tile([128, 1], BF16, tag="ones_col")
        nc.vector.memset(ones_col, 1.0)
        ones_row = persist.tile([1, 128], BF16, tag="ones_row")
        nc.vector.memset(ones_row, 1.0)
        id128 = persist.tile([128, 128], BF16, tag="id128")
        make_identity(nc, id128)

        qkvT = [qkv_pool.tile([128, N], BF16, name=f"qkvT{jt}", tag=f"qkvT{jt}")
                for jt in range(12)]
        concatT = [concat_pool.tile([128, N], BF16, name=f"concatT{fc}",
                                    tag=f"concatT{fc}") for fc in range(4)]

        # W layouts (single wide tiles, chunk-major):
        #   wq_big[cp, cc*1536 + jt*128 + s]   (lhsT chunks for qkv)
        #   wp_big[fp, fc*512 + e]             (rhs chunks for proj)
        wq_big = persist.tile([128, 4 * 1536], BF16, tag="wq_big")
        wp_big = persist.tile([128, 4 * 512], BF16, tag="wp_big")

        # ---------------- prologue ----------------
        with tc.tile_pool(name="xload", bufs=3) as xload, \
             tc.tile_pool(name="xcast", bufs=3) as xcast, \
             tc.tile_pool(name="xtg", bufs=3) as xtg_pool, \
             tc.tile_pool(name="qkv_psum", bufs=4, space="PSUM") as qkv_psum, \
             tc.tile_pool(name="w_psum", bufs=3, space="PSUM") as w_psum:

            def pe_transpose_block(wrow_bf, out_tile, base, blk_stride):
                """4x4 [128,128] PE transposes: out[:, base + cc*blk_stride
                + j*128 + s] = wrow_bf[:, j, cc*128+s].T, via bf16 PSUM."""
                for cc in range(4):
                    ps = w_psum.tile([128, 512], BF16, tag="xps")
                    for j in range(4):
                        nc.tensor.transpose(
                            ps[:, j * 128:(j + 1) * 128],
                            wrow_bf[:, j, cc * 128:(cc + 1) * 128], id128)
                    dst = bass.AP(
                        tensor=out_tile.tensor,
                        offset=out_tile.offset + base + cc * blk_stride,
                        ap=[out_tile.ap[0], [1, 512]])
                    if cc % 2 == 0:
                        nc.scalar.activation(dst, ps, ACT.Copy)
                    else:
                        nc.vector.tensor_copy(dst, ps)

            # prefetch the first x chunk before the W loads
            xrow0 = xload.tile([128, 4, C], F32, tag="xrow")
            nc.sync.dma_start(
                out=xrow0,
                in_=x_d.rearrange("(a p) c -> p a c", p=128)[:, 0:4, :])

            # --- Wqkv ---
            for jg in range(3):
                wrow = xload.tile([128, 4, C], F32, tag="xrow")
                nc.sync.dma_start(
                    out=wrow,
                    in_=wqkv_d.rearrange("(a p) c -> p a c", p=128)[
                        :, jg * 4:(jg + 1) * 4, :])
                wrow_bf = xcast.tile([128, 4, C], BF16, tag="xrow_bf")
                for j in range(4):
                    if j % 2 == 0:
                        nc.vector.tensor_copy(wrow_bf[:, j], wrow[:, j])
                    else:
                        nc.scalar.activation(wrow_bf[:, j], wrow[:, j], ACT.Copy)
                pe_transpose_block(wrow_bf, wq_big, jg * 512, 1536)

            # --- biases ---
            bqkv_sb = persist.tile([128, 12], F32, tag="bqkv")
            nc.sync.dma_start(out=bqkv_sb,
                              in_=bqkv_d.rearrange("(a p) -> p a", p=128))
            # --- x chunks: load -> cast -> PE transpose -> qkv matmuls ---
            for tg in range(8):
                if tg == 0:
                    xrow = xrow0
                else:
                    xrow = xload.tile([128, 4, C], F32, tag="xrow")
                    nc.sync.dma_start(
                        out=xrow,
                        in_=x_d.rearrange("(a p) c -> p a c", p=128)[
                            :, tg * 4:(tg + 1) * 4, :])
                xrow_bf = xcast.tile([128, 4, C], BF16, tag="xrow_bf")
                for j in range(4):
                    if j % 2 == 0:
                        nc.vector.tensor_copy(xrow_bf[:, j], xrow[:, j])
                    else:
                        nc.scalar.activation(xrow_bf[:, j], xrow[:, j], ACT.Copy)
                # xTg[cp, cc*512 + j*128 + tsub] for this token chunk
                xTg = xtg_pool.tile([128, 2048], BF16, tag="xTg", name="xTg")
                pe_transpose_block(xrow_bf, xTg, 0, 512)
                for jt in range(12):
                    vhalf = (jt % 4) >= 2
                    ps = qkv_psum.tile([128, 512], F32, tag="qkvps")
                    for cc in range(4):
                        nc.tensor.matmul(
                            ps,
                            wq_big[:, cc * 1536 + jt * 128:
                                   cc * 1536 + jt * 128 + 128],
                            xTg[:, cc * 512:(cc + 1) * 512],
                            start=(cc == 0), stop=(cc == 3))
                    if vhalf:
                        out_ap = bass.AP(
                            tensor=qkvT[jt].tensor,
                            offset=qkvT[jt].offset + 8 * tg,
                            ap=[qkvT[jt].ap[0], [1, 8], [64, 64]])
                        nc.vector.tensor_scalar_add(
                            out_ap, ps, bqkv_sb[:, jt:jt + 1])
                    else:
                        nc.scalar.activation(
                            qkvT[jt][:, tg * 512:(tg + 1) * 512], ps,
                            ACT.Identity, bias=bqkv_sb[:, jt:jt + 1])

            # --- Wproj ---
            wrow = xload.tile([128, 4, C], F32, tag="xrow")
            nc.sync.dma_start(
                out=wrow, in_=wproj_d.rearrange("(a p) c -> p a c", p=128))
            wrow_bf = xcast.tile([128, 4, C], BF16, tag="xrow_bf")
            for j in range(4):
                if j % 2 == 0:
                    nc.vector.tensor_copy(wrow_bf[:, j], wrow[:, j])
                else:
                    nc.scalar.activation(wrow_bf[:, j], wrow[:, j], ACT.Copy)
            pe_transpose_block(wrow_bf, wp_big, 0, 512)

            bproj_row = persist.tile([1, C], F32, tag="bproj_row")
            nc.sync.dma_start(out=bproj_row,
                              in_=bproj_d.rearrange("(a e) -> a e", a=1))
            bproj_sb = persist.tile([1, C], BF16, tag="bproj_sb")
            nc.vector.tensor_copy(bproj_sb, bproj_row)
            bproj_bc = persist.tile([128, C], F32, tag="bproj_bc")
            nc.gpsimd.partition_broadcast(bproj_bc, bproj_row)
            lepe_b = []
            for name, d in (("lhb", lhb_d), ("lvb", lvb_d)):
                t = persist.tile([128, 1], F32, name=name, tag=name)
                nc.sync.dma_start(out=t[0:64, :],
                                  in_=d.rearrange("(p a) -> p a", a=1))
                nc.sync.dma_start(out=t[64:128, :],
                                  in_=d.rearrange("(p a) -> p a", a=1))
                lepe_b.append(t)

            # --- LePE weights ---
            lepw = []
            diags = []
            for half, wsrc in ((0, lhw_d), (1, lvw_d)):
                w9 = xload.tile([9, 64], F32, tag="w9")
                nc.sync.dma_start(out=w9,
                                  in_=wsrc.rearrange("a b c d -> (a b c) d"))
                w9_bf = xcast.tile([9, 64], BF16, tag="w9bf")
                nc.vector.tensor_copy(w9_bf, w9)
                ps = w_psum.tile([64, 9], BF16, tag="wTps", bufs=1)
                nc.tensor.transpose(ps, w9_bf, id64[0:9, 0:9])
                wT = persist.tile([128, 9], F32, tag=f"wT{half}")
                nc.vector.tensor_copy(wT[0:64, :], ps)
                nc.sync.dma_start(out=wT[64:128, :], in_=wT[0:64, :])
                wis = []
                for k in range(9):
                    dr, dc = k // 3 - 1, k % 3 - 1
                    wis.append((dr + 1) * 3 + (dc + 1) if half == 0
                               else (dc + 1) * 3 + (dr + 1))
                wic = wis[4]
                nc.vector.tensor_scalar_add(wT[:, wic:wic + 1],
                                            wT[:, wic:wic + 1], 1.0)
                lepw.append((wT, wis))
                dh = {}
                for k in PE_TAPS:
                    dt = persist.tile([128, 128], BF16, tag=f"diag{half}_{k}")
                    nc.vector.tensor_scalar_mul(dt, id128,
                                                wT[:, wis[k]:wis[k] + 1])
                    dh[k] = dt
                diags.append(dh)


        # ---------------- attention ----------------
        pairs = [(half, s, hp)
                 for half in range(2) for s in range(NS) for hp in range(2)]

        with tc.tile_pool(name="sc_psum", bufs=2, space="PSUM") as sc_psum, \
             tc.tile_pool(name="lp_psum", bufs=2, space="PSUM") as lp_psum, \
             tc.tile_pool(name="oa_psum", bufs=2, space="PSUM") as oa_psum, \
             tc.tile_pool(name="att", bufs=8) as att, \
             tc.tile_pool(name="lepe_sb", bufs=3) as lepe_sb, \
             tc.tile_pool(name="norm_sb", bufs=6) as norm_sb, \
             tc.tile_pool(name="ttp", bufs=4) as ttp_pool, \
             tc.tile_pool(name="vcm", bufs=3) as vcm_pool:

            def vsp_of(pi):
                half, s, hp = pairs[pi]
                return qkvT[8 + half * 2 + hp][:, s * 512:s * 512 + 512]

            def emit_pe_taps(pi):
                """center + 5 taps as diagonal matmuls in PSUM."""
                half, s, hp = pairs[pi]
                vsp = vsp_of(pi)
                vl = lp_psum.tile([128, 512], F32, tag="lps", name="vl")
                nc.tensor.matmul(vl, diags[half][4], vsp,
                                 start=True, stop=False, skip_group_check=True)
                for ki, k in enumerate(PE_TAPS[1:]):
                    o_off, i_off, nr, nx = _geom(k)
                    nc.tensor.matmul(
                        _win(vl, o_off, nr, nx), diags[half][k],
                        _win(vsp, i_off, nr, nx),
                        start=False, stop=(ki == len(PE_TAPS) - 2),
                        skip_group_check=True)
                return vl

            def emit_dve_chain(pi):
                """taps 3,5,7 in-place on DVE; first tap initializes."""
                half, s, hp = pairs[pi]
                vsp = vsp_of(pi)
                wT, wis = lepw[half]
                acc = lepe_sb.tile([128, 512], F32, tag="acc", name="acc")
                k0 = DVE_TAPS[0]
                o_off, i_off, nr, nx = _geom(k0)
                nc.vector.tensor_scalar_mul(
                    _win(acc, o_off, nr, nx), _win(vsp, i_off, nr, nx),
                    wT[:, wis[k0]:wis[k0] + 1])
                # complement of tap 1's window (rows 1..7): row 0
                nc.vector.memset(acc[:, 0:64], 0.0)
                for k in DVE_TAPS[1:]:
                    o_off, i_off, nr, nx = _geom(k)
                    nc.vector.scalar_tensor_tensor(
                        _win(acc, o_off, nr, nx), _win(vsp, i_off, nr, nx),
                        wT[:, wis[k]:wis[k] + 1], _win(acc, o_off, nr, nx),
                        ALU.mult, ALU.add)
                return acc

            def emit_combine(pi, vl, acc):
                half, s, hp = pairs[pi]
                vlsb = lepe_sb.tile([128, 512], BF16, tag="vlsb", name="vlsb")
                nc.vector.scalar_tensor_tensor(
                    vlsb, vl, lepe_b[half], acc, ALU.add, ALU.add)
                return vlsb

            def emit_vna(vlsb):
                vna = lepe_sb.tile([128, 4, 128], BF16, tag="vna", name="vna")
                nc.sync.dma_start_transpose(vna, vlsb)
                return vna

            def emit_scores(pi):
                half, s, hp = pairs[pi]
                tok0 = s * 512
                jt_off = half * 2 + hp
                esbs = []
                for hh in range(2):
                    esbs.append(att.tile([128, 2048], BF16, tag="esb",
                                         name="esb"))
                for sh in range(2):
                    for hh in range(2):
                        pbase = hh * 64
                        qs = qkvT[jt_off][pbase:pbase + 64, tok0:tok0 + 512]
                        ks = qkvT[4 + jt_off][pbase:pbase + 64, tok0:tok0 + 512]
                        sps = sc_psum.tile([128, 1024], F32, tag="sps",
                                           name="sps")
                        for jj in range(2):
                            jc = 2 * sh + jj
                            nc.tensor.matmul(
                                sps[:, jj * 512:(jj + 1) * 512],
                                ks[:, jc * 128:(jc + 1) * 128], qs,
                                start=True, stop=True)
                        nc.scalar.activation(
                            esbs[hh][:, sh * 1024:(sh + 1) * 1024], sps,
                            ACT.Exp, bias=0.0, scale=SCALE)
                return esbs

            def emit_av(hh, vna, esb):
                """flipped AV matmuls -> oa [128, 260] (one PSUM bank)."""
                oa = oa_psum.tile([128, 260], F32, tag="oa", name="oa")
                first = True
                for u in range(4):
                    for jc in range(4):
                        lhsT = esb[:, jc * 512 + u * 128:
                                   jc * 512 + u * 128 + 128]
                        nc.tensor.matmul(
                            oa[:, u * 65:u * 65 + 64], lhsT,
                            vna[:, jc, hh * 64:hh * 64 + 64],
                            start=first, stop=False, skip_group_check=True)
                        first = False
                        nc.tensor.matmul(
                            oa[:, u * 65 + 64:u * 65 + 65], lhsT, ones_col,
                            start=False, stop=(u == 3 and jc == 3),
                            skip_group_check=True)
                return oa

            def emit_norm(hh, oa, ttp):
                """batched reciprocal + 0-stride broadcast normalize into the
                pair's token-major collect tile (cols u*128 + 64*hh)."""
                rr = norm_sb.tile([128, 4], F32, tag="rr", name="rr")
                nc.vector.reciprocal(
                    rr, bass.AP(tensor=oa.tensor, offset=oa.offset + 64,
                                ap=[oa.ap[0], [65, 4]]))
                oa_data = bass.AP(tensor=oa.tensor, offset=oa.offset,
                                  ap=[oa.ap[0], [65, 4], [1, 64]])
                rr_b = bass.AP(tensor=rr.tensor, offset=rr.offset,
                               ap=[rr.ap[0], [1, 4], [0, 64]])
                out_ap = bass.AP(
                    tensor=ttp.tensor, offset=ttp.offset + 64 * hh,
                    ap=[ttp.ap[0], [128, 4], [1, 64]])
                nc.vector.tensor_tensor(out_ap, oa_data, rr_b, ALU.mult)

            def emit_assembly(pi, ttp):
                """pair's collect tile -> concatT (channel-major)."""
                half, s, hp = pairs[pi]
                fc = half * 2 + hp
                if half == 0:
                    out_ap = bass.AP(
                        tensor=concatT[fc].tensor,
                        offset=concatT[fc].offset + s * 512,
                        ap=[concatT[fc].ap[0], [128, 4], [1, 128]])
                    nc.sync.dma_start_transpose(out_ap, ttp)
                else:
                    vcm = vcm_pool.tile([128, 4, 128], BF16, tag="vcm",
                                        name="vcm")
                    nc.sync.dma_start_transpose(vcm, ttp)
                    out_ap = bass.AP(
                        tensor=concatT[fc].tensor,
                        offset=concatT[fc].offset + 8 * s,
                        ap=[concatT[fc].ap[0], [1, 8], [64, 64]])
                    nc.vector.tensor_copy(
                        out_ap, vcm.rearrange("p a b -> p (a b)"))

            # steady-state loop; LePE/vna AND scores pipelined 1 pair ahead
            streams = [list(range(0, 16)), list(range(16, 32))]
            vna_cur = [None, None]
            esb_cur = [None, None]
            for st in (0, 1):
                p0 = streams[st][0]
                vl = emit_pe_taps(p0)
                acc = emit_dve_chain(p0)
                vna_cur[st] = emit_vna(emit_combine(p0, vl, acc))
                esb_cur[st] = emit_scores(p0)
            nsteps = len(streams[0])
            for i in range(nsteps):
                p0, p1 = streams[0][i], streams[1][i]
                n0 = streams[0][i + 1] if i + 1 < nsteps else None
                n1 = streams[1][i + 1] if i + 1 < nsteps else None
                esbA, esbB = esb_cur
                # PE: NEXT pairs' scores first (their exps run this step).
                # DVE queue order: current norms lead (freeing oa banks for
                # PE), next-pair chains fill the middle, combines close.
                esbA_n = emit_scores(n0) if n0 is not None else None
                esbB_n = emit_scores(n1) if n1 is not None else None
                ttp0 = ttp_pool.tile([128, 512], BF16, tag="ttp", name="ttp")
                oa = emit_av(0, vna_cur[0], esbA[0])
                emit_norm(0, oa, ttp0)
                oa = emit_av(1, vna_cur[0], esbA[1])
                emit_norm(1, oa, ttp0)
                acc0 = emit_dve_chain(n0) if n0 is not None else None
                emit_assembly(p0, ttp0)
                vl0 = emit_pe_taps(n0) if n0 is not None else None
                ttp1 = ttp_pool.tile([128, 512], BF16, tag="ttp", name="ttp")
                oa = emit_av(0, vna_cur[1], esbB[0])
                emit_norm(0, oa, ttp1)
                oa = emit_av(1, vna_cur[1], esbB[1])
                emit_norm(1, oa, ttp1)
                acc1 = emit_dve_chain(n1) if n1 is not None else None
                emit_assembly(p1, ttp1)
                vl1 = emit_pe_taps(n1) if n1 is not None else None
                if n0 is not None:
                    vna_cur[0] = emit_vna(emit_combine(n0, vl0, acc0))
                if n1 is not None:
                    vna_cur[1] = emit_vna(emit_combine(n1, vl1, acc1))
                esb_cur = [esbA_n, esbB_n]

        # ---------------- proj ----------------
        with tc.tile_pool(name="pj_psum", bufs=4, space="PSUM") as pj_psum, \
             tc.tile_pool(name="pj", bufs=3) as pj:
            osb = None
            for tt in range(32):
                even = (tt % 2 == 0)
                ps = pj_psum.tile([128, C], F32, tag="pjps")
                for fcc in range(4):
                    nc.tensor.matmul(
                        ps, concatT[fcc][:, tt * 128:(tt + 1) * 128],
                        wp_big[:, fcc * 512:(fcc + 1) * 512],
                        start=(fcc == 0), stop=(fcc == 3),
                        skip_group_check=True)
                if even:
                    osb = pj.tile([128, 2, C], F32, tag="pjout", name="pjout")
                # bias folded into the DVE PSUM->SBUF add (no K=1 matmul)
                nc.vector.tensor_tensor(osb[:, tt % 2], ps, bproj_bc, ALU.add)
                if not even:
                    nc.sync.dma_start(
                        out=y_d.rearrange("(a p) c -> p a c", p=128)[
                            :, tt - 1:tt + 1, :],
                        in_=osb)


def _get_nc():
    if "nc" not in _CACHE:
        _CACHE["nc"] = _build_nc()
    return _CACHE["nc"]


def kernel(**inputs):
    x = np.asarray(inputs["x"], dtype=np.float32)
    names = {
        "wqkv": "Wqkv", "bqkv": "bqkv", "wproj": "Wproj", "bproj": "bproj",
        "lepe_h_w": "lepe_h_w", "lepe_h_b": "lepe_h_b",
        "lepe_v_w": "lepe_v_w", "lepe_v_b": "lepe_v_b",
    }
    shared = {k: np.ascontiguousarray(np.asarray(inputs[v], dtype=np.float32))
              for k, v in names.items()}
    nc = _get_nc()
    in_maps = []
    for b in range(B):
        m = dict(shared)
        m["x"] = np.ascontiguousarray(x[b])
        in_maps.append(m)
    res = bass_utils.run_bass_kernel_spmd(nc, in_maps, core_ids=list(range(B)))
    out = np.stack([res.results[b]["y"] for b in range(B)], axis=0)
    return out.astype(np.float32)


if __name__ == "__main__":
    rng = np.random.default_rng(0)
    ins = {
        "x": rng.standard_normal((B, N, C), dtype=np.float32),
        "Wqkv": rng.standard_normal((3 * C, C), dtype=np.float32) * C ** -0.5,
        "bqkv": np.zeros(3 * C, np.float32),
        "Wproj": rng.standard_normal((C, C), dtype=np.float32) * C ** -0.5,
        "bproj": np.zeros(C, np.float32),
        "lepe_h_w": rng.standard_normal((3, 3, 1, HD), dtype=np.float32) / 3,
        "lepe_h_b": np.zeros(HD, np.float32),
        "lepe_v_w": rng.standard_normal((3, 3, 1, HD), dtype=np.float32) / 3,
        "lepe_v_b": np.zeros(HD, np.float32),
        "H": np.int64(H), "W": np.int64(W),
    }
    out = kernel(**ins)
    print(out.shape, out.dtype)


# revision 26
# speedup vs baseline: 1.0151x; 1.0025x over previous
"""CSWin attention Trainium2 kernel (v6 — flipped AV, split LePE, XBAR
attention transposes, in-loop concat assembly).

Shapes (hardcoded): B=8, H=W=64, N=4096, C=512, 8 heads (4 horizontal-stripe,
4 vertical-stripe), head_dim=64, stripe width SPLIT=8.

Sharding: data-parallel over batch B across the 8 NeuronCores (1 image/core).

Per-core structure (all matmuls bf16, fp32 PSUM):
  - prologue: batched f32 DMA loads (first x chunk prefetched ahead of the
    weights); f32->bf16 casts split DVE/ScalarE; x/W transposes as PE
    [128,128] transposes through bf16 PSUM (kept on PE: XBAR versions
    serialize on the issuing sequencer and add latency to the qkv
    pipeline); per-512-token chunk: 12 qkv matmuls + PSUM->SBUF bias
    copies (h-half on ScalarE via Identity+per-partition bias, v-half
    scattered col-major on DVE).
  - attention: 32 head-pairs in two interleaved streams (h / v), LePE AND
    scores both software-pipelined one pair ahead.  LePE split: center +
    4 corner taps as diagonal matmuls on PE (center first: a PSUM zero
    region gets exactly one start=True and it must cover the bank), 4
    edge taps as a DVE in-place scalar_tensor_tensor chain; one DVE op
    combines PSUM + chain + bias into bf16.  v_lepe goes token-major via
    XBAR DMA transpose (14 ns per 16x128 tile on the idle DMA engines).
    AV is flipped: out[tq, u*65] packs 4 tq-chunks in one PSUM bank and
    uses all 128 PE output partitions (65 cols/matmul instead of 512);
    interleaved 1-col ones-matmuls produce the softmax denominator in a
    per-partition column.  Normalization: batched DVE reciprocal [128,4]
    + one 0-stride-broadcast tensor_tensor per head-stripe into a small
    token-major collect tile.  Assembly runs in-loop per pair: XBAR
    transpose to channel-major concatT (v-half through a col-major
    staging tile + DVE scatter-copy for the global token order).
  - proj: 32 chunks; bias via K=1 ones-matmul on even chunks (ScalarE
    copy-out) and folded into the DVE PSUM->SBUF add on odd chunks.
"""

import numpy as np

import concourse.bass as bass
import concourse.bacc as bacc
import concourse.mybir as mybir
from concourse import bass_utils
from concourse.tile import TileContext
from concourse.masks import make_identity

F32 = mybir.dt.float32
BF16 = mybir.dt.bfloat16
ALU = mybir.AluOpType
ACT = mybir.ActivationFunctionType

B = 8
H = 64
W = 64
N = H * W          # 4096
C = 512
NH = 8             # heads
HD = 64            # head dim
SP = 8             # stripe width
NS = 8             # stripes per direction
SCALE = HD ** -0.5

# LePE tap split: PE gets center (full width, owns the PSUM zero region),
# all 4 corners and the k=1 edge; DVE chain gets taps 3, 5, 7 (first one
# initializes the accumulator; its complement is memset to 0).
PE_TAPS = (4, 0, 2, 6, 8)
DVE_TAPS = (1, 3, 5, 7)

_CACHE = {}


def _win(t, off, nr, nx):
    return bass.AP(tensor=t.tensor, offset=t.offset + off,
                   ap=[t.ap[0], [64, nr], [1, nx]])


def _geom(k):
    dr, dc = k // 3 - 1, k % 3 - 1
    r0 = max(0, -dr)
    nr = 8 - abs(dr)
    x0 = max(0, -dc)
    nx = 64 - abs(dc)
    return r0 * 64 + x0, (r0 + dr) * 64 + (x0 + dc), nr, nx


def _build_nc():
    nc = bacc.Bacc("TRN2", target_bir_lowering=False, debug=False)

    x_d = nc.dram_tensor("x", (N, C), F32, kind="ExternalInput").ap()
    wqkv_d = nc.dram_tensor("wqkv", (3 * C, C), F32, kind="ExternalInput").ap()
    bqkv_d = nc.dram_tensor("bqkv", (3 * C,), F32, kind="ExternalInput").ap()
    wproj_d = nc.dram_tensor("wproj", (C, C), F32, kind="ExternalInput").ap()
    bproj_d = nc.dram_tensor("bproj", (C,), F32, kind="ExternalInput").ap()
    lhw_d = nc.dram_tensor("lepe_h_w", (3, 3, 1, HD), F32, kind="ExternalInput").ap()
    lhb_d = nc.dram_tensor("lepe_h_b", (HD,), F32, kind="ExternalInput").ap()
    lvw_d = nc.dram_tensor("lepe_v_w", (3, 3, 1, HD), F32, kind="ExternalInput").ap()
    lvb_d = nc.dram_tensor("lepe_v_b", (HD,), F32, kind="ExternalInput").ap()
    y_d = nc.dram_tensor("y", (N, C), F32, kind="ExternalOutput").ap()

    with TileContext(nc) as tc:
        _emit(nc, tc, x_d, wqkv_d, bqkv_d, wproj_d, bproj_d,
              lhw_d, lhb_d, lvw_d, lvb_d, y_d)
    nc.compile()
    return nc


def _emit(nc, tc, x_d, wqkv_d, bqkv_d, wproj_d, bproj_d,
          lhw_d, lhb_d, lvw_d, lvb_d, y_d):
    import contextlib
    ctx = contextlib.ExitStack()
    with ctx:
        persist = ctx.enter_context(tc.tile_pool(name="persist", bufs=1))
        qkv_pool = ctx.enter_context(tc.tile_pool(name="qkvT", bufs=1))
        concat_pool = ctx.enter_context(tc.tile_pool(name="concat", bufs=1))

        from concourse import library_config
        nc.gpsimd.load_library(library_config.proxy)

        # ---------------- constants ----------------
        id64 = persist.tile([64, 64], BF16, tag="id64")
        make_identity(nc, id64)
        ones_col = persist.tile([128, 1], BF16, tag="ones_col")
        nc.vector.memset(ones_col, 1.0)
        ones_row = persist.tile([1, 128], BF16, tag="ones_row")
        nc.vector.memset(ones_row, 1.0)
        id128 = persist.tile([128, 128], BF16, tag="id128")
        make_identity(nc, id128)

        qkvT = [qkv_pool.tile([128, N], BF16, name=f"qkvT{jt}", tag=f"qkvT{jt}")
                for jt in range(12)]
        concatT = [concat_pool.tile([128, N], BF16, name=f"concatT{fc}",
                                    tag=f"concatT{fc}") for fc in range(4)]

        # W layouts (single wide tiles, chunk-major):
        #   wq_big[cp, cc*1536 + jt*128 + s]   (lhsT chunks for qkv)
        #   wp_big[fp, fc*512 + e]             (rhs chunks for proj)
        wq_big = persist.tile([128, 4 * 1536], BF16, tag="wq_big")
        wp_big = persist.tile([128, 4 * 512], BF16, tag="wp_big")

        # ---------------- prologue ----------------
        with tc.tile_pool(name="xload", bufs=3) as xload, \
             tc.tile_pool(name="xcast", bufs=3) as xcast, \
             tc.tile_pool(name="xtg", bufs=3) as xtg_pool, \
             tc.tile_pool(name="qkv_psum", bufs=4, space="PSUM") as qkv_psum, \
             tc.tile_pool(name="w_psum", bufs=3, space="PSUM") as w_psum:

            def pe_transpose_block(wrow_bf, out_tile, base, blk_stride):
                """4x4 [128,128] PE transposes: out[:, base + cc*blk_stride
                + j*128 + s] = wrow_bf[:, j, cc*128+s].T, via bf16 PSUM."""
                for cc in range(4):
                    ps = w_psum.tile([128, 512], BF16, tag="xps")
                    for j in range(4):
                        nc.tensor.transpose(
                            ps[:, j * 128:(j + 1) * 128],
                            wrow_bf[:, j, cc * 128:(cc + 1) * 128], id128)
                    dst = bass.AP(
                        tensor=out_tile.tensor,
                        offset=out_tile.offset + base + cc * blk_stride,
                        ap=[out_tile.ap[0], [1, 512]])
                    if cc % 2 == 0:
                        nc.scalar.activation(dst, ps, ACT.Copy)
                    else:
                        nc.vector.tensor_copy(dst, ps)

            # prefetch the first x chunk before the W loads (two half
            # DMAs so the first casts start sooner)
            xrow0 = xload.tile([128, 4, C], F32, tag="xrow")
            nc.sync.dma_start(
                out=xrow0[:, 0:2],
                in_=x_d.rearrange("(a p) c -> p a c", p=128)[:, 0:2, :])
            nc.sync.dma_start(
                out=xrow0[:, 2:4],
                in_=x_d.rearrange("(a p) c -> p a c", p=128)[:, 2:4, :])

            # --- Wqkv --- (first group split in half for a faster ramp)
            for jg in range(3):
                wrow = xload.tile([128, 4, C], F32, tag="xrow")
                if jg == 0:
                    nc.sync.dma_start(
                        out=wrow[:, 0:2],
                        in_=wqkv_d.rearrange("(a p) c -> p a c", p=128)[
                            :, 0:2, :])
                    nc.sync.dma_start(
                        out=wrow[:, 2:4],
                        in_=wqkv_d.rearrange("(a p) c -> p a c", p=128)[
                            :, 2:4, :])
                else:
                    nc.sync.dma_start(
                        out=wrow,
                        in_=wqkv_d.rearrange("(a p) c -> p a c", p=128)[
                            :, jg * 4:(jg + 1) * 4, :])
                wrow_bf = xcast.tile([128, 4, C], BF16, tag="xrow_bf")
                for j in range(4):
                    if j % 2 == 0:
                        nc.vector.tensor_copy(wrow_bf[:, j], wrow[:, j])
                    else:
                        nc.scalar.activation(wrow_bf[:, j], wrow[:, j], ACT.Copy)
                pe_transpose_block(wrow_bf, wq_big, jg * 512, 1536)

            # --- biases ---
            bqkv_sb = persist.tile([128, 12], F32, tag="bqkv")
            nc.sync.dma_start(out=bqkv_sb,
                              in_=bqkv_d.rearrange("(a p) -> p a", p=128))
            # --- x chunks: load -> cast -> PE transpose -> qkv matmuls ---
            for tg in range(8):
                if tg == 0:
                    xrow = xrow0
                else:
                    xrow = xload.tile([128, 4, C], F32, tag="xrow")
                    nc.sync.dma_start(
                        out=xrow,
                        in_=x_d.rearrange("(a p) c -> p a c", p=128)[
                            :, tg * 4:(tg + 1) * 4, :])
                xrow_bf = xcast.tile([128, 4, C], BF16, tag="xrow_bf")
                for j in range(4):
                    if j % 2 == 0:
                        nc.vector.tensor_copy(xrow_bf[:, j], xrow[:, j])
                    else:
                        nc.scalar.activation(xrow_bf[:, j], xrow[:, j], ACT.Copy)
                # xTg[cp, cc*512 + j*128 + tsub] for this token chunk
                xTg = xtg_pool.tile([128, 2048], BF16, tag="xTg", name="xTg")
                pe_transpose_block(xrow_bf, xTg, 0, 512)
                for jt in range(12):
                    vhalf = (jt % 4) >= 2
                    ps = qkv_psum.tile([128, 512], F32, tag="qkvps")
                    for cc in range(4):
                        nc.tensor.matmul(
                            ps,
                            wq_big[:, cc * 1536 + jt * 128:
                                   cc * 1536 + jt * 128 + 128],
                            xTg[:, cc * 512:(cc + 1) * 512],
                            start=(cc == 0), stop=(cc == 3))
                    if vhalf:
                        out_ap = bass.AP(
                            tensor=qkvT[jt].tensor,
                            offset=qkvT[jt].offset + 8 * tg,
                            ap=[qkvT[jt].ap[0], [1, 8], [64, 64]])
                        nc.vector.tensor_scalar_add(
                            out_ap, ps, bqkv_sb[:, jt:jt + 1])
                    else:
                        nc.scalar.activation(
                            qkvT[jt][:, tg * 512:(tg + 1) * 512], ps,
                            ACT.Identity, bias=bqkv_sb[:, jt:jt + 1])

            # --- Wproj ---
            wrow = xload.tile([128, 4, C], F32, tag="xrow")
            nc.sync.dma_start(
                out=wrow, in_=wproj_d.rearrange("(a p) c -> p a c", p=128))
            wrow_bf = xcast.tile([128, 4, C], BF16, tag="xrow_bf")
            for j in range(4):
                if j % 2 == 0:
                    nc.vector.tensor_copy(wrow_bf[:, j], wrow[:, j])
                else:
                    nc.scalar.activation(wrow_bf[:, j], wrow[:, j], ACT.Copy)
            pe_transpose_block(wrow_bf, wp_big, 0, 512)

            bproj_row = persist.tile([1, C], F32, tag="bproj_row")
            nc.sync.dma_start(out=bproj_row,
                              in_=bproj_d.rearrange("(a e) -> a e", a=1))
            bproj_sb = persist.tile([1, C], BF16, tag="bproj_sb")
            nc.vector.tensor_copy(bproj_sb, bproj_row)
            bproj_bc = persist.tile([128, C], F32, tag="bproj_bc")
            nc.gpsimd.partition_broadcast(bproj_bc, bproj_row)
            lepe_b = []
            for name, d in (("lhb", lhb_d), ("lvb", lvb_d)):
                t = persist.tile([128, 1], F32, name=name, tag=name)
                nc.sync.dma_start(out=t[0:64, :],
                                  in_=d.rearrange("(p a) -> p a", a=1))
                nc.sync.dma_start(out=t[64:128, :],
                                  in_=d.rearrange("(p a) -> p a", a=1))
                lepe_b.append(t)

            # --- LePE weights ---
            lepw = []
            diags = []
            for half, wsrc in ((0, lhw_d), (1, lvw_d)):
                w9 = xload.tile([9, 64], F32, tag="w9")
                nc.sync.dma_start(out=w9,
                                  in_=wsrc.rearrange("a b c d -> (a b c) d"))
                w9_bf = xcast.tile([9, 64], BF16, tag="w9bf")
                nc.vector.tensor_copy(w9_bf, w9)
                ps = w_psum.tile([64, 9], BF16, tag="wTps", bufs=1)
                nc.tensor.transpose(ps, w9_bf, id64[0:9, 0:9])
                wT = persist.tile([128, 9], F32, tag=f"wT{half}")
                nc.vector.tensor_copy(wT[0:64, :], ps)
                nc.sync.dma_start(out=wT[64:128, :], in_=wT[0:64, :])
                wis = []
                for k in range(9):
                    dr, dc = k // 3 - 1, k % 3 - 1
                    wis.append((dr + 1) * 3 + (dc + 1) if half == 0
                               else (dc + 1) * 3 + (dr + 1))
                wic = wis[4]
                nc.vector.tensor_scalar_add(wT[:, wic:wic + 1],
                                            wT[:, wic:wic + 1], 1.0)
                lepw.append((wT, wis))
                dh = {}
                for k in PE_TAPS:
                    dt = persist.tile([128, 128], BF16, tag=f"diag{half}_{k}")
                    nc.vector.tensor_scalar_mul(dt, id128,
                                                wT[:, wis[k]:wis[k] + 1])
                    dh[k] = dt
                diags.append(dh)


        # ---------------- attention ----------------
        pairs = [(half, s, hp)
                 for half in range(2) for s in range(NS) for hp in range(2)]

        with tc.tile_pool(name="sc_psum", bufs=2, space="PSUM") as sc_psum, \
             tc.tile_pool(name="lp_psum", bufs=2, space="PSUM") as lp_psum, \
             tc.tile_pool(name="oa_psum", bufs=2, space="PSUM") as oa_psum, \
             tc.tile_pool(name="att", bufs=8) as att, \
             tc.tile_pool(name="lepe_sb", bufs=3) as lepe_sb, \
             tc.tile_pool(name="norm_sb", bufs=6) as norm_sb, \
             tc.tile_pool(name="ttp", bufs=4) as ttp_pool, \
             tc.tile_pool(name="vcm", bufs=3) as vcm_pool:

            def vsp_of(pi):
                half, s, hp = pairs[pi]
                return qkvT[8 + half * 2 + hp][:, s * 512:s * 512 + 512]

            def emit_pe_taps(pi):
                """center + 5 taps as diagonal matmuls in PSUM."""
                half, s, hp = pairs[pi]
                vsp = vsp_of(pi)
                vl = lp_psum.tile([128, 512], F32, tag="lps", name="vl")
                nc.tensor.matmul(vl, diags[half][4], vsp,
                                 start=True, stop=False, skip_group_check=True)
                for ki, k in enumerate(PE_TAPS[1:]):
                    o_off, i_off, nr, nx = _geom(k)
                    nc.tensor.matmul(
                        _win(vl, o_off, nr, nx), diags[half][k],
                        _win(vsp, i_off, nr, nx),
                        start=False, stop=(ki == len(PE_TAPS) - 2),
                        skip_group_check=True)
                return vl

            def emit_dve_chain(pi):
                """taps 3,5,7 in-place on DVE; first tap initializes."""
                half, s, hp = pairs[pi]
                vsp = vsp_of(pi)
                wT, wis = lepw[half]
                acc = lepe_sb.tile([128, 512], F32, tag="acc", name="acc")
                k0 = DVE_TAPS[0]
                o_off, i_off, nr, nx = _geom(k0)
                nc.vector.tensor_scalar_mul(
                    _win(acc, o_off, nr, nx), _win(vsp, i_off, nr, nx),
                    wT[:, wis[k0]:wis[k0] + 1])
                # complement of tap 1's window (rows 1..7): row 0
                nc.vector.memset(acc[:, 0:64], 0.0)
                for k in DVE_TAPS[1:]:
                    o_off, i_off, nr, nx = _geom(k)
                    nc.vector.scalar_tensor_tensor(
                        _win(acc, o_off, nr, nx), _win(vsp, i_off, nr, nx),
                        wT[:, wis[k]:wis[k] + 1], _win(acc, o_off, nr, nx),
                        ALU.mult, ALU.add)
                return acc

            def emit_combine(pi, vl, acc):
                half, s, hp = pairs[pi]
                vlsb = lepe_sb.tile([128, 512], BF16, tag="vlsb", name="vlsb")
                nc.vector.scalar_tensor_tensor(
                    vlsb, vl, lepe_b[half], acc, ALU.add, ALU.add)
                return vlsb

            def emit_vna(vlsb):
                vna = lepe_sb.tile([128, 4, 128], BF16, tag="vna", name="vna")
                nc.sync.dma_start_transpose(vna, vlsb)
                return vna

            def emit_scores(pi):
                half, s, hp = pairs[pi]
                tok0 = s * 512
                jt_off = half * 2 + hp
                esbs = []
                for hh in range(2):
                    esbs.append(att.tile([128, 2048], BF16, tag="esb",
                                         name="esb"))
                for sh in range(2):
                    for hh in range(2):
                        pbase = hh * 64
                        qs = qkvT[jt_off][pbase:pbase + 64, tok0:tok0 + 512]
                        ks = qkvT[4 + jt_off][pbase:pbase + 64, tok0:tok0 + 512]
                        sps = sc_psum.tile([128, 1024], F32, tag="sps",
                                           name="sps")
                        for jj in range(2):
                            jc = 2 * sh + jj
                            nc.tensor.matmul(
                                sps[:, jj * 512:(jj + 1) * 512],
                                ks[:, jc * 128:(jc + 1) * 128], qs,
                                start=True, stop=True)
                        nc.scalar.activation(
                            esbs[hh][:, sh * 1024:(sh + 1) * 1024], sps,
                            ACT.Exp, bias=0.0, scale=SCALE)
                return esbs

            def emit_av(hh, vna, esb):
                """flipped AV matmuls -> oa [128, 260] (one PSUM bank)."""
                oa = oa_psum.tile([128, 260], F32, tag="oa", name="oa")
                first = True
                for u in range(4):
                    for jc in range(4):
                        lhsT = esb[:, jc * 512 + u * 128:
                                   jc * 512 + u * 128 + 128]
                        nc.tensor.matmul(
                            oa[:, u * 65:u * 65 + 64], lhsT,
                            vna[:, jc, hh * 64:hh * 64 + 64],
                            start=first, stop=False, skip_group_check=True)
                        first = False
                        nc.tensor.matmul(
                            oa[:, u * 65 + 64:u * 65 + 65], lhsT, ones_col,
                            start=False, stop=(u == 3 and jc == 3),
                            skip_group_check=True)
                return oa

            def emit_norm(hh, oa, ttp):
                """batched reciprocal + 0-stride broadcast normalize into the
                pair's token-major collect tile (cols u*128 + 64*hh)."""
                rr = norm_sb.tile([128, 4], F32, tag="rr", name="rr")
                nc.vector.reciprocal(
                    rr, bass.AP(tensor=oa.tensor, offset=oa.offset + 64,
                                ap=[oa.ap[0], [65, 4]]))
                oa_data = bass.AP(tensor=oa.tensor, offset=oa.offset,
                                  ap=[oa.ap[0], [65, 4], [1, 64]])
                rr_b = bass.AP(tensor=rr.tensor, offset=rr.offset,
                               ap=[rr.ap[0], [1, 4], [0, 64]])
                out_ap = bass.AP(
                    tensor=ttp.tensor, offset=ttp.offset + 64 * hh,
                    ap=[ttp.ap[0], [128, 4], [1, 64]])
                nc.vector.tensor_tensor(out_ap, oa_data, rr_b, ALU.mult)

            def emit_assembly(pi, ttp):
                """pair's collect tile -> concatT (channel-major)."""
                half, s, hp = pairs[pi]
                fc = half * 2 + hp
                if half == 0:
                    out_ap = bass.AP(
                        tensor=concatT[fc].tensor,
                        offset=concatT[fc].offset + s * 512,
                        ap=[concatT[fc].ap[0], [128, 4], [1, 128]])
                    nc.sync.dma_start_transpose(out_ap, ttp)
                else:
                    vcm = vcm_pool.tile([128, 4, 128], BF16, tag="vcm",
                                        name="vcm")
                    nc.sync.dma_start_transpose(vcm, ttp)
                    out_ap = bass.AP(
                        tensor=concatT[fc].tensor,
                        offset=concatT[fc].offset + 8 * s,
                        ap=[concatT[fc].ap[0], [1, 8], [64, 64]])
                    nc.vector.tensor_copy(
                        out_ap, vcm.rearrange("p a b -> p (a b)"))

            # steady-state loop; LePE/vna AND scores pipelined 1 pair ahead
            streams = [list(range(0, 16)), list(range(16, 32))]
            vna_cur = [None, None]
            esb_cur = [None, None]
            for st in (0, 1):
                p0 = streams[st][0]
                vl = emit_pe_taps(p0)
                acc = emit_dve_chain(p0)
                vna_cur[st] = emit_vna(emit_combine(p0, vl, acc))
                esb_cur[st] = emit_scores(p0)
            nsteps = len(streams[0])
            for i in range(nsteps):
                p0, p1 = streams[0][i], streams[1][i]
                n0 = streams[0][i + 1] if i + 1 < nsteps else None
                n1 = streams[1][i + 1] if i + 1 < nsteps else None
                esbA, esbB = esb_cur
                # PE: NEXT pairs' scores first (their exps run this step).
                # DVE queue order: current norms lead (freeing oa banks for
                # PE), next-pair chains fill the middle, combines close.
                esbA_n = emit_scores(n0) if n0 is not None else None
                esbB_n = emit_scores(n1) if n1 is not None else None
                ttp0 = ttp_pool.tile([128, 512], BF16, tag="ttp", name="ttp")
                oa = emit_av(0, vna_cur[0], esbA[0])
                emit_norm(0, oa, ttp0)
                oa = emit_av(1, vna_cur[0], esbA[1])
                emit_norm(1, oa, ttp0)
                acc0 = emit_dve_chain(n0) if n0 is not None else None
                emit_assembly(p0, ttp0)
                vl0 = emit_pe_taps(n0) if n0 is not None else None
                ttp1 = ttp_pool.tile([128, 512], BF16, tag="ttp", name="ttp")
                oa = emit_av(0, vna_cur[1], esbB[0])
                emit_norm(0, oa, ttp1)
                oa = emit_av(1, vna_cur[1], esbB[1])
                emit_norm(1, oa, ttp1)
                acc1 = emit_dve_chain(n1) if n1 is not None else None
                emit_assembly(p1, ttp1)
                vl1 = emit_pe_taps(n1) if n1 is not None else None
                if n0 is not None:
                    vna_cur[0] = emit_vna(emit_combine(n0, vl0, acc0))
                if n1 is not None:
                    vna_cur[1] = emit_vna(emit_combine(n1, vl1, acc1))
                esb_cur = [esbA_n, esbB_n]

        # ---------------- proj ----------------
        with tc.tile_pool(name="pj_psum", bufs=4, space="PSUM") as pj_psum, \
             tc.tile_pool(name="pj", bufs=3) as pj:
            osb = None
            for tt in range(32):
                even = (tt % 2 == 0)
                ps = pj_psum.tile([128, C], F32, tag="pjps")
                for fcc in range(4):
                    nc.tensor.matmul(
                        ps, concatT[fcc][:, tt * 128:(tt + 1) * 128],
                        wp_big[:, fcc * 512:(fcc + 1) * 512],
                        start=(fcc == 0), stop=(fcc == 3),
                        skip_group_check=True)
                if even:
                    osb = pj.tile([128, 2, C], F32, tag="pjout", name="pjout")
                # bias folded into the DVE PSUM->SBUF add (no K=1 matmul)
                nc.vector.tensor_tensor(osb[:, tt % 2], ps, bproj_bc, ALU.add)
                if not even:
                    nc.sync.dma_start(
                        out=y_d.rearrange("(a p) c -> p a c", p=128)[
                            :, tt - 1:tt + 1, :],
                        in_=osb)


def _get_nc():
    if "nc" not in _CACHE:
        _CACHE["nc"] = _build_nc()
    return _CACHE["nc"]


def kernel(**inputs):
    x = np.asarray(inputs["x"], dtype=np.float32)
    names = {
        "wqkv": "Wqkv", "bqkv": "bqkv", "wproj": "Wproj", "bproj": "bproj",
        "lepe_h_w": "lepe_h_w", "lepe_h_b": "lepe_h_b",
        "lepe_v_w": "lepe_v_w", "lepe_v_b": "lepe_v_b",
    }
    shared = {k: np.ascontiguousarray(np.asarray(inputs[v], dtype=np.float32))
              for k, v in names.items()}
    nc = _get_nc()
    in_maps = []
    for b in range(B):
        m = dict(shared)
        m["x"] = np.ascontiguousarray(x[b])
        in_maps.append(m)
    res = bass_utils.run_bass_kernel_spmd(nc, in_maps, core_ids=list(range(B)))
    out = np.stack([res.results[b]["y"] for b in range(B)], axis=0)
    return out.astype(np.float32)


if __name__ == "__main__":
    rng = np.random.default_rng(0)
    ins = {
        "x": rng.standard_normal((B, N, C), dtype=np.float32),
        "Wqkv": rng.standard_normal((3 * C, C), dtype=np.float32) * C ** -0.5,
        "bqkv": np.zeros(3 * C, np.float32),
        "Wproj": rng.standard_normal((C, C), dtype=np.float32) * C ** -0.5,
        "bproj": np.zeros(C, np.float32),
        "lepe_h_w": rng.standard_normal((3, 3, 1, HD), dtype=np.float32) / 3,
        "lepe_h_b": np.zeros(HD, np.float32),
        "lepe_v_w": rng.standard_normal((3, 3, 1, HD), dtype=np.float32) / 3,
        "lepe_v_b": np.zeros(HD, np.float32),
        "H": np.int64(H), "W": np.int64(W),
    }
    out = kernel(**ins)
    print(out.shape, out.dtype)


# revision 27
# speedup vs baseline: 1.0274x; 1.0121x over previous
"""CSWin attention Trainium2 kernel (v6 — flipped AV, split LePE, XBAR
attention transposes, in-loop concat assembly).

Shapes (hardcoded): B=8, H=W=64, N=4096, C=512, 8 heads (4 horizontal-stripe,
4 vertical-stripe), head_dim=64, stripe width SPLIT=8.

Sharding: data-parallel over batch B across the 8 NeuronCores (1 image/core).

Per-core structure (all matmuls bf16, fp32 PSUM):
  - prologue: batched f32 DMA loads (first x chunk prefetched ahead of the
    weights); f32->bf16 casts split DVE/ScalarE; x/W transposes as PE
    [128,128] transposes through bf16 PSUM (kept on PE: XBAR versions
    serialize on the issuing sequencer and add latency to the qkv
    pipeline); per-512-token chunk: 12 qkv matmuls + PSUM->SBUF bias
    copies (h-half on ScalarE via Identity+per-partition bias, v-half
    scattered col-major on DVE).
  - attention: 32 head-pairs in two interleaved streams (h / v), LePE AND
    scores both software-pipelined one pair ahead.  LePE split: center +
    4 corner taps as diagonal matmuls on PE (center first: a PSUM zero
    region gets exactly one start=True and it must cover the bank), 4
    edge taps as a DVE in-place scalar_tensor_tensor chain; one DVE op
    combines PSUM + chain + bias into bf16.  v_lepe goes token-major via
    XBAR DMA transpose (14 ns per 16x128 tile on the idle DMA engines).
    AV is flipped: out[tq, u*65] packs 4 tq-chunks in one PSUM bank and
    uses all 128 PE output partitions (65 cols/matmul instead of 512);
    interleaved 1-col ones-matmuls produce the softmax denominator in a
    per-partition column.  Normalization: batched DVE reciprocal [128,4]
    + one 0-stride-broadcast tensor_tensor per head-stripe into a small
    token-major collect tile.  Assembly runs in-loop per pair: XBAR
    transpose to channel-major concatT (v-half through a col-major
    staging tile + DVE scatter-copy for the global token order).
  - proj: 32 chunks; bias via K=1 ones-matmul on even chunks (ScalarE
    copy-out) and folded into the DVE PSUM->SBUF add on odd chunks.
"""

import numpy as np

import concourse.bass as bass
import concourse.bacc as bacc
import concourse.mybir as mybir
from concourse import bass_utils
from concourse.tile import TileContext
from concourse.masks import make_identity

F32 = mybir.dt.float32
BF16 = mybir.dt.bfloat16
ALU = mybir.AluOpType
ACT = mybir.ActivationFunctionType

B = 8
H = 64
W = 64
N = H * W          # 4096
C = 512
NH = 8             # heads
HD = 64            # head dim
SP = 8             # stripe width
NS = 8             # stripes per direction
SCALE = HD ** -0.5

# LePE tap split: PE gets center (full width, owns the PSUM zero region),
# all 4 corners and the k=1 edge; DVE chain gets taps 3, 5, 7 (first one
# initializes the accumulator; its complement is memset to 0).
PE_TAPS = (4, 0, 2, 6, 8)
DVE_TAPS = (1, 3, 5, 7)

_CACHE = {}


def _win(t, off, nr, nx):
    return bass.AP(tensor=t.tensor, offset=t.offset + off,
                   ap=[t.ap[0], [64, nr], [1, nx]])


def _geom(k):
    dr, dc = k // 3 - 1, k % 3 - 1
    r0 = max(0, -dr)
    nr = 8 - abs(dr)
    x0 = max(0, -dc)
    nx = 64 - abs(dc)
    return r0 * 64 + x0, (r0 + dr) * 64 + (x0 + dc), nr, nx


def _build_nc():
    nc = bacc.Bacc("TRN2", target_bir_lowering=False, debug=False)

    x_d = nc.dram_tensor("x", (N, C), F32, kind="ExternalInput").ap()
    wqkv_d = nc.dram_tensor("wqkv", (3 * C, C), F32, kind="ExternalInput").ap()
    bqkv_d = nc.dram_tensor("bqkv", (3 * C,), F32, kind="ExternalInput").ap()
    wproj_d = nc.dram_tensor("wproj", (C, C), F32, kind="ExternalInput").ap()
    bproj_d = nc.dram_tensor("bproj", (C,), F32, kind="ExternalInput").ap()
    lhw_d = nc.dram_tensor("lepe_h_w", (3, 3, 1, HD), F32, kind="ExternalInput").ap()
    lhb_d = nc.dram_tensor("lepe_h_b", (HD,), F32, kind="ExternalInput").ap()
    lvw_d = nc.dram_tensor("lepe_v_w", (3, 3, 1, HD), F32, kind="ExternalInput").ap()
    lvb_d = nc.dram_tensor("lepe_v_b", (HD,), F32, kind="ExternalInput").ap()
    y_d = nc.dram_tensor("y", (N, C), F32, kind="ExternalOutput").ap()

    with TileContext(nc) as tc:
        _emit(nc, tc, x_d, wqkv_d, bqkv_d, wproj_d, bproj_d,
              lhw_d, lhb_d, lvw_d, lvb_d, y_d)
    nc.compile()
    return nc


def _emit(nc, tc, x_d, wqkv_d, bqkv_d, wproj_d, bproj_d,
          lhw_d, lhb_d, lvw_d, lvb_d, y_d):
    import contextlib
    ctx = contextlib.ExitStack()
    with ctx:
        persist = ctx.enter_context(tc.tile_pool(name="persist", bufs=1))
        qkv_pool = ctx.enter_context(tc.tile_pool(name="qkvT", bufs=1))
        concat_pool = ctx.enter_context(tc.tile_pool(name="concat", bufs=1))

        from concourse import library_config
        nc.gpsimd.load_library(library_config.proxy)

        # ---------------- constants ----------------
        id64 = persist.tile([64, 64], BF16, tag="id64")
        make_identity(nc, id64)
        ones_col = persist.tile([128, 1], BF16, tag="ones_col")
        nc.vector.memset(ones_col, 1.0)
        ones_row = persist.tile([1, 128], BF16, tag="ones_row")
        nc.vector.memset(ones_row, 1.0)
        id128 = persist.tile([128, 128], BF16, tag="id128")
        make_identity(nc, id128)

        qkvT = [qkv_pool.tile([128, N], BF16, name=f"qkvT{jt}", tag=f"qkvT{jt}")
                for jt in range(12)]
        concatT = [concat_pool.tile([128, N], BF16, name=f"concatT{fc}",
                                    tag=f"concatT{fc}") for fc in range(4)]

        # W layouts (single wide tiles, chunk-major):
        #   wq_big[cp, cc*1536 + jt*128 + s]   (lhsT chunks for qkv)
        #   wp_big[fp, fc*512 + e]             (rhs chunks for proj)
        wq_big = persist.tile([128, 4 * 1536], BF16, tag="wq_big")
        wp_big = persist.tile([128, 4 * 512], BF16, tag="wp_big")

        # ---------------- prologue ----------------
        with tc.tile_pool(name="xload", bufs=3) as xload, \
             tc.tile_pool(name="xcast", bufs=3) as xcast, \
             tc.tile_pool(name="xtg", bufs=3) as xtg_pool, \
             tc.tile_pool(name="qkv_psum", bufs=4, space="PSUM") as qkv_psum, \
             tc.tile_pool(name="w_psum", bufs=3, space="PSUM") as w_psum:

            def pe_transpose_block(wrow_bf, out_tile, base, blk_stride):
                """4x4 [128,128] PE transposes: out[:, base + cc*blk_stride
                + j*128 + s] = wrow_bf[:, j, cc*128+s].T, via bf16 PSUM."""
                for cc in range(4):
                    ps = w_psum.tile([128, 512], BF16, tag="xps")
                    for j in range(4):
                        nc.tensor.transpose(
                            ps[:, j * 128:(j + 1) * 128],
                            wrow_bf[:, j, cc * 128:(cc + 1) * 128], id128)
                    dst = bass.AP(
                        tensor=out_tile.tensor,
                        offset=out_tile.offset + base + cc * blk_stride,
                        ap=[out_tile.ap[0], [1, 512]])
                    if cc % 2 == 0:
                        nc.scalar.activation(dst, ps, ACT.Copy)
                    else:
                        nc.vector.tensor_copy(dst, ps)

            # prefetch the first x chunk before the W loads (two half
            # DMAs so the first casts start sooner)
            xrow0 = xload.tile([128, 4, C], F32, tag="xrow")
            nc.sync.dma_start(
                out=xrow0[:, 0:2],
                in_=x_d.rearrange("(a p) c -> p a c", p=128)[:, 0:2, :])
            nc.sync.dma_start(
                out=xrow0[:, 2:4],
                in_=x_d.rearrange("(a p) c -> p a c", p=128)[:, 2:4, :])

            # --- Wqkv --- (first group split in half for a faster ramp)
            for jg in range(3):
                wrow = xload.tile([128, 4, C], F32, tag="xrow")
                if jg == 0:
                    nc.sync.dma_start(
                        out=wrow[:, 0:2],
                        in_=wqkv_d.rearrange("(a p) c -> p a c", p=128)[
                            :, 0:2, :])
                    nc.sync.dma_start(
                        out=wrow[:, 2:4],
                        in_=wqkv_d.rearrange("(a p) c -> p a c", p=128)[
                            :, 2:4, :])
                else:
                    nc.sync.dma_start(
                        out=wrow,
                        in_=wqkv_d.rearrange("(a p) c -> p a c", p=128)[
                            :, jg * 4:(jg + 1) * 4, :])
                wrow_bf = xcast.tile([128, 4, C], BF16, tag="xrow_bf")
                for j in range(4):
                    if j % 2 == 0:
                        nc.vector.tensor_copy(wrow_bf[:, j], wrow[:, j])
                    else:
                        nc.scalar.activation(wrow_bf[:, j], wrow[:, j], ACT.Copy)
                pe_transpose_block(wrow_bf, wq_big, jg * 512, 1536)

            # --- biases ---
            bqkv_sb = persist.tile([128, 12], F32, tag="bqkv")
            nc.sync.dma_start(out=bqkv_sb,
                              in_=bqkv_d.rearrange("(a p) -> p a", p=128))
            # --- x chunks: load -> cast -> PE transpose -> qkv matmuls ---
            for tg in range(8):
                if tg == 0:
                    xrow = xrow0
                else:
                    xrow = xload.tile([128, 4, C], F32, tag="xrow")
                    nc.sync.dma_start(
                        out=xrow,
                        in_=x_d.rearrange("(a p) c -> p a c", p=128)[
                            :, tg * 4:(tg + 1) * 4, :])
                xrow_bf = xcast.tile([128, 4, C], BF16, tag="xrow_bf")
                for j in range(4):
                    if j % 2 == 0:
                        nc.vector.tensor_copy(xrow_bf[:, j], xrow[:, j])
                    else:
                        nc.scalar.activation(xrow_bf[:, j], xrow[:, j], ACT.Copy)
                # xTg[cp, cc*512 + j*128 + tsub] for this token chunk
                xTg = xtg_pool.tile([128, 2048], BF16, tag="xTg", name="xTg")
                pe_transpose_block(xrow_bf, xTg, 0, 512)
                for jt in range(12):
                    vhalf = (jt % 4) >= 2
                    ps = qkv_psum.tile([128, 512], F32, tag="qkvps")
                    for cc in range(4):
                        nc.tensor.matmul(
                            ps,
                            wq_big[:, cc * 1536 + jt * 128:
                                   cc * 1536 + jt * 128 + 128],
                            xTg[:, cc * 512:(cc + 1) * 512],
                            start=(cc == 0), stop=(cc == 3))
                    if vhalf:
                        out_ap = bass.AP(
                            tensor=qkvT[jt].tensor,
                            offset=qkvT[jt].offset + 8 * tg,
                            ap=[qkvT[jt].ap[0], [1, 8], [64, 64]])
                        nc.vector.tensor_scalar_add(
                            out_ap, ps, bqkv_sb[:, jt:jt + 1])
                    else:
                        nc.scalar.activation(
                            qkvT[jt][:, tg * 512:(tg + 1) * 512], ps,
                            ACT.Identity, bias=bqkv_sb[:, jt:jt + 1])

            # --- Wproj ---
            wrow = xload.tile([128, 4, C], F32, tag="xrow")
            nc.sync.dma_start(
                out=wrow, in_=wproj_d.rearrange("(a p) c -> p a c", p=128))
            wrow_bf = xcast.tile([128, 4, C], BF16, tag="xrow_bf")
            for j in range(4):
                if j % 2 == 0:
                    nc.vector.tensor_copy(wrow_bf[:, j], wrow[:, j])
                else:
                    nc.scalar.activation(wrow_bf[:, j], wrow[:, j], ACT.Copy)
            pe_transpose_block(wrow_bf, wp_big, 0, 512)

            bproj_row = persist.tile([1, C], F32, tag="bproj_row")
            nc.sync.dma_start(out=bproj_row,
                              in_=bproj_d.rearrange("(a e) -> a e", a=1))
            bproj_sb = persist.tile([1, C], BF16, tag="bproj_sb")
            nc.vector.tensor_copy(bproj_sb, bproj_row)
            bproj_bc = persist.tile([128, C], F32, tag="bproj_bc")
            nc.gpsimd.partition_broadcast(bproj_bc, bproj_row)
            lepe_b = []
            for name, d in (("lhb", lhb_d), ("lvb", lvb_d)):
                t = persist.tile([128, 1], F32, name=name, tag=name)
                nc.sync.dma_start(out=t[0:64, :],
                                  in_=d.rearrange("(p a) -> p a", a=1))
                nc.sync.dma_start(out=t[64:128, :],
                                  in_=d.rearrange("(p a) -> p a", a=1))
                lepe_b.append(t)

            # --- LePE weights ---
            lepw = []
            diags = []
            for half, wsrc in ((0, lhw_d), (1, lvw_d)):
                w9 = xload.tile([9, 64], F32, tag="w9")
                nc.sync.dma_start(out=w9,
                                  in_=wsrc.rearrange("a b c d -> (a b c) d"))
                w9_bf = xcast.tile([9, 64], BF16, tag="w9bf")
                nc.vector.tensor_copy(w9_bf, w9)
                ps = w_psum.tile([64, 9], BF16, tag="wTps", bufs=1)
                nc.tensor.transpose(ps, w9_bf, id64[0:9, 0:9])
                wT = persist.tile([128, 9], F32, tag=f"wT{half}")
                nc.vector.tensor_copy(wT[0:64, :], ps)
                nc.sync.dma_start(out=wT[64:128, :], in_=wT[0:64, :])
                wis = []
                for k in range(9):
                    dr, dc = k // 3 - 1, k % 3 - 1
                    wis.append((dr + 1) * 3 + (dc + 1) if half == 0
                               else (dc + 1) * 3 + (dr + 1))
                wic = wis[4]
                nc.vector.tensor_scalar_add(wT[:, wic:wic + 1],
                                            wT[:, wic:wic + 1], 1.0)
                lepw.append((wT, wis))
                dh = {}
                for k in PE_TAPS:
                    dt = persist.tile([128, 128], BF16, tag=f"diag{half}_{k}")
                    nc.vector.tensor_scalar_mul(dt, id128,
                                                wT[:, wis[k]:wis[k] + 1])
                    dh[k] = dt
                diags.append(dh)


        # ---------------- attention ----------------
        pairs = [(half, s, hp)
                 for half in range(2) for s in range(NS) for hp in range(2)]

        with tc.tile_pool(name="sc_psum", bufs=2, space="PSUM") as sc_psum, \
             tc.tile_pool(name="lp_psum", bufs=2, space="PSUM") as lp_psum, \
             tc.tile_pool(name="oa_psum", bufs=2, space="PSUM") as oa_psum, \
             tc.tile_pool(name="att", bufs=8) as att, \
             tc.tile_pool(name="lepe_sb", bufs=3) as lepe_sb, \
             tc.tile_pool(name="norm_sb", bufs=6) as norm_sb, \
             tc.tile_pool(name="ttp", bufs=4) as ttp_pool, \
             tc.tile_pool(name="vcm", bufs=3) as vcm_pool:

            def vsp_of(pi):
                half, s, hp = pairs[pi]
                return qkvT[8 + half * 2 + hp][:, s * 512:s * 512 + 512]

            def emit_pe_taps(pi):
                """center + 5 taps as diagonal matmuls in PSUM."""
                half, s, hp = pairs[pi]
                vsp = vsp_of(pi)
                vl = lp_psum.tile([128, 512], F32, tag="lps", name="vl")
                nc.tensor.matmul(vl, diags[half][4], vsp,
                                 start=True, stop=False, skip_group_check=True)
                for ki, k in enumerate(PE_TAPS[1:]):
                    o_off, i_off, nr, nx = _geom(k)
                    nc.tensor.matmul(
                        _win(vl, o_off, nr, nx), diags[half][k],
                        _win(vsp, i_off, nr, nx),
                        start=False, stop=(ki == len(PE_TAPS) - 2),
                        skip_group_check=True)
                return vl

            def emit_dve_chain(pi):
                """taps 3,5,7 in-place on DVE; first tap initializes."""
                half, s, hp = pairs[pi]
                vsp = vsp_of(pi)
                wT, wis = lepw[half]
                acc = lepe_sb.tile([128, 512], F32, tag="acc", name="acc")
                k0 = DVE_TAPS[0]
                o_off, i_off, nr, nx = _geom(k0)
                nc.vector.tensor_scalar_mul(
                    _win(acc, o_off, nr, nx), _win(vsp, i_off, nr, nx),
                    wT[:, wis[k0]:wis[k0] + 1])
                # complement of tap 1's window (rows 1..7): row 0
                nc.vector.memset(acc[:, 0:64], 0.0)
                for k in DVE_TAPS[1:]:
                    o_off, i_off, nr, nx = _geom(k)
                    nc.vector.scalar_tensor_tensor(
                        _win(acc, o_off, nr, nx), _win(vsp, i_off, nr, nx),
                        wT[:, wis[k]:wis[k] + 1], _win(acc, o_off, nr, nx),
                        ALU.mult, ALU.add)
                return acc

            def emit_combine(pi, vl, acc):
                half, s, hp = pairs[pi]
                vlsb = lepe_sb.tile([128, 512], BF16, tag="vlsb", name="vlsb")
                nc.vector.scalar_tensor_tensor(
                    vlsb, vl, lepe_b[half], acc, ALU.add, ALU.add)
                return vlsb

            def emit_vna(vlsb):
                vna = lepe_sb.tile([128, 4, 128], BF16, tag="vna", name="vna")
                nc.sync.dma_start_transpose(vna, vlsb)
                return vna

            def emit_scores(pi):
                half, s, hp = pairs[pi]
                tok0 = s * 512
                jt_off = half * 2 + hp
                esbs = []
                for hh in range(2):
                    esbs.append(att.tile([128, 2048], BF16, tag="esb",
                                         name="esb"))
                for sh in range(2):
                    for hh in range(2):
                        pbase = hh * 64
                        qs = qkvT[jt_off][pbase:pbase + 64, tok0:tok0 + 512]
                        ks = qkvT[4 + jt_off][pbase:pbase + 64, tok0:tok0 + 512]
                        sps = sc_psum.tile([128, 1024], F32, tag="sps",
                                           name="sps")
                        for jj in range(2):
                            jc = 2 * sh + jj
                            nc.tensor.matmul(
                                sps[:, jj * 512:(jj + 1) * 512],
                                ks[:, jc * 128:(jc + 1) * 128], qs,
                                start=True, stop=True)
                        nc.scalar.activation(
                            esbs[hh][:, sh * 1024:(sh + 1) * 1024], sps,
                            ACT.Exp, bias=0.0, scale=SCALE)
                return esbs

            def emit_av(hh, vna, esb):
                """flipped AV matmuls -> oa [128, 260] (one PSUM bank)."""
                oa = oa_psum.tile([128, 260], F32, tag="oa", name="oa")
                first = True
                for u in range(4):
                    for jc in range(4):
                        lhsT = esb[:, jc * 512 + u * 128:
                                   jc * 512 + u * 128 + 128]
                        nc.tensor.matmul(
                            oa[:, u * 65:u * 65 + 64], lhsT,
                            vna[:, jc, hh * 64:hh * 64 + 64],
                            start=first, stop=False, skip_group_check=True)
                        first = False
                        nc.tensor.matmul(
                            oa[:, u * 65 + 64:u * 65 + 65], lhsT, ones_col,
                            start=False, stop=(u == 3 and jc == 3),
                            skip_group_check=True)
                return oa

            def emit_norm(hh, oa, ttp):
                """batched reciprocal + 0-stride broadcast normalize into the
                pair's token-major collect tile (cols u*128 + 64*hh)."""
                rr = norm_sb.tile([128, 4], F32, tag="rr", name="rr")
                nc.vector.reciprocal(
                    rr, bass.AP(tensor=oa.tensor, offset=oa.offset + 64,
                                ap=[oa.ap[0], [65, 4]]))
                oa_data = bass.AP(tensor=oa.tensor, offset=oa.offset,
                                  ap=[oa.ap[0], [65, 4], [1, 64]])
                rr_b = bass.AP(tensor=rr.tensor, offset=rr.offset,
                               ap=[rr.ap[0], [1, 4], [0, 64]])
                out_ap = bass.AP(
                    tensor=ttp.tensor, offset=ttp.offset + 64 * hh,
                    ap=[ttp.ap[0], [128, 4], [1, 64]])
                nc.vector.tensor_tensor(out_ap, oa_data, rr_b, ALU.mult)

            def emit_assembly(pi, ttp):
                """pair's collect tile -> concatT (channel-major)."""
                half, s, hp = pairs[pi]
                fc = half * 2 + hp
                if half == 0:
                    out_ap = bass.AP(
                        tensor=concatT[fc].tensor,
                        offset=concatT[fc].offset + s * 512,
                        ap=[concatT[fc].ap[0], [128, 4], [1, 128]])
                    nc.sync.dma_start_transpose(out_ap, ttp)
                else:
                    vcm = vcm_pool.tile([128, 4, 128], BF16, tag="vcm",
                                        name="vcm")
                    nc.sync.dma_start_transpose(vcm, ttp)
                    out_ap = bass.AP(
                        tensor=concatT[fc].tensor,
                        offset=concatT[fc].offset + 8 * s,
                        ap=[concatT[fc].ap[0], [1, 8], [64, 64]])
                    nc.vector.tensor_copy(
                        out_ap, vcm.rearrange("p a b -> p (a b)"))

            # steady-state loop; LePE/vna AND scores pipelined 1 pair ahead
            streams = [list(range(0, 16)), list(range(16, 32))]
            vna_cur = [None, None]
            esb_cur = [None, None]
            for st in (0, 1):
                p0 = streams[st][0]
                vl = emit_pe_taps(p0)
                acc = emit_dve_chain(p0)
                vna_cur[st] = emit_vna(emit_combine(p0, vl, acc))
                esb_cur[st] = emit_scores(p0)
            nsteps = len(streams[0])
            for i in range(nsteps):
                p0, p1 = streams[0][i], streams[1][i]
                n0 = streams[0][i + 1] if i + 1 < nsteps else None
                n1 = streams[1][i + 1] if i + 1 < nsteps else None
                esbA, esbB = esb_cur
                # PE: NEXT pairs' scores first (their exps run this step).
                # DVE queue order: current norms lead (freeing oa banks for
                # PE), next-pair chains fill the middle, combines close.
                esbA_n = emit_scores(n0) if n0 is not None else None
                esbB_n = emit_scores(n1) if n1 is not None else None
                ttp0 = ttp_pool.tile([128, 512], BF16, tag="ttp", name="ttp")
                oa = emit_av(0, vna_cur[0], esbA[0])
                emit_norm(0, oa, ttp0)
                oa = emit_av(1, vna_cur[0], esbA[1])
                emit_norm(1, oa, ttp0)
                acc0 = emit_dve_chain(n0) if n0 is not None else None
                emit_assembly(p0, ttp0)
                vl0 = emit_pe_taps(n0) if n0 is not None else None
                ttp1 = ttp_pool.tile([128, 512], BF16, tag="ttp", name="ttp")
                oa = emit_av(0, vna_cur[1], esbB[0])
                emit_norm(0, oa, ttp1)
                oa = emit_av(1, vna_cur[1], esbB[1])
                emit_norm(1, oa, ttp1)
                acc1 = emit_dve_chain(n1) if n1 is not None else None
                emit_assembly(p1, ttp1)
                vl1 = emit_pe_taps(n1) if n1 is not None else None
                if n0 is not None:
                    vna_cur[0] = emit_vna(emit_combine(n0, vl0, acc0))
                if n1 is not None:
                    vna_cur[1] = emit_vna(emit_combine(n1, vl1, acc1))
                esb_cur = [esbA_n, esbB_n]

        # ---------------- proj ----------------
        with tc.tile_pool(name="pj_psum", bufs=6, space="PSUM") as pj_psum, \
             tc.tile_pool(name="pj", bufs=4) as pj:
            osb = None
            for tt in range(32):
                even = (tt % 2 == 0)
                ps = pj_psum.tile([128, C], F32, tag="pjps")
                for fcc in range(4):
                    nc.tensor.matmul(
                        ps, concatT[fcc][:, tt * 128:(tt + 1) * 128],
                        wp_big[:, fcc * 512:(fcc + 1) * 512],
                        start=(fcc == 0), stop=(fcc == 3),
                        skip_group_check=True)
                if even:
                    osb = pj.tile([128, 2, C], F32, tag="pjout", name="pjout")
                # bias folded into the DVE PSUM->SBUF add (no K=1 matmul)
                nc.vector.tensor_tensor(osb[:, tt % 2], ps, bproj_bc, ALU.add)
                if not even:
                    nc.sync.dma_start(
                        out=y_d.rearrange("(a p) c -> p a c", p=128)[
                            :, tt - 1:tt + 1, :],
                        in_=osb)


def _get_nc():
    if "nc" not in _CACHE:
        _CACHE["nc"] = _build_nc()
    return _CACHE["nc"]


def kernel(**inputs):
    x = np.asarray(inputs["x"], dtype=np.float32)
    names = {
        "wqkv": "Wqkv", "bqkv": "bqkv", "wproj": "Wproj", "bproj": "bproj",
        "lepe_h_w": "lepe_h_w", "lepe_h_b": "lepe_h_b",
        "lepe_v_w": "lepe_v_w", "lepe_v_b": "lepe_v_b",
    }
    shared = {k: np.ascontiguousarray(np.asarray(inputs[v], dtype=np.float32))
              for k, v in names.items()}
    nc = _get_nc()
    in_maps = []
    for b in range(B):
        m = dict(shared)
        m["x"] = np.ascontiguousarray(x[b])
        in_maps.append(m)
    res = bass_utils.run_bass_kernel_spmd(nc, in_maps, core_ids=list(range(B)))
    out = np.stack([res.results[b]["y"] for b in range(B)], axis=0)
    return out.astype(np.float32)


if __name__ == "__main__":
    rng = np.random.default_rng(0)
    ins = {
        "x": rng.standard_normal((B, N, C), dtype=np.float32),
        "Wqkv": rng.standard_normal((3 * C, C), dtype=np.float32) * C ** -0.5,
        "bqkv": np.zeros(3 * C, np.float32),
        "Wproj": rng.standard_normal((C, C), dtype=np.float32) * C ** -0.5,
        "bproj": np.zeros(C, np.float32),
        "lepe_h_w": rng.standard_normal((3, 3, 1, HD), dtype=np.float32) / 3,
        "lepe_h_b": np.zeros(HD, np.float32),
        "lepe_v_w": rng.standard_normal((3, 3, 1, HD), dtype=np.float32) / 3,
        "lepe_v_b": np.zeros(HD, np.float32),
        "H": np.int64(H), "W": np.int64(W),
    }
    out = kernel(**ins)
    print(out.shape, out.dtype)
